# revision 15
# baseline (speedup 1.0000x reference)
"""CoAttLayer Trainium2 kernel — pure data-parallel over batch on 8 NeuronCores.

Reference computation (per batch element b, T=1024, N=512, D=64, K=80):
  L  = tanh(R @ Wl @ P^T)                    (T, N)
  Hp = tanh(Wp @ P^T + (Wr @ R^T) @ L)       (K, N)
  Hr = tanh(Wr @ R^T + (Wp @ P^T) @ L^T)     (K, T)
  Ap = softmax(whp @ Hp), Ar = softmax(whr @ Hr)
  out[b] = concat(P^T @ Ap, R^T @ Ar)        (2D,)

Reassociated into D-sized contractions:
  Hp = [Wp | Wr] @ [P^T ; X]   with X = R^T @ L    (D, N)
  Hr = [Wr | Wp] @ [R^T ; Y]   with Y = P^T @ L^T  (D, T)

Design notes (from trace analysis):
 - The PE HAM clock governor only counts real matmul activity; transpose-mode
   instructions poison it back to 1.2 GHz. So the batch loop contains ZERO PE
   transposes: all static transposed layouts (R^T, P^T, weight stacks) are
   prepared on the HOST, and the data-dependent L^T is produced by bouncing
   L through DRAM and reading it back through the DMA xbar transpose engine
   (~180 GB/s, fully off the compute engines).
 - All matmul operands are bf16 (fp32 PSUM accumulate); tanh lives on the
   Scalar engine with 1024-wide evacuations; PSUM evacuations go to DVE.
 - Softmax is batched across the 8 local batch elements on partitions.
"""

import numpy as np

import concourse.bass as bass
import concourse.bacc as bacc
import concourse.mybir as mybir
import concourse.tile as tile
from concourse.bass_utils import run_bass_kernel_spmd

F32 = mybir.dt.float32
BF16 = mybir.dt.bfloat16
AF = mybir.ActivationFunctionType

B_LOC = 8      # batch elements per core
T, N, D, K = 1024, 512, 64, 80
TI = T // 128  # 8 t-tiles
NI = N // 128  # 4 n-tiles
NCORES = 8


def build_kernel():
    nc = bacc.Bacc("TRN2", debug=False, target_bir_lowering=False)

    ins = {}
    for name, shape, dt in [
        ("review_bf", [B_LOC, T, D], BF16),
        ("review_t", [B_LOC, D, T], BF16),
        ("post_bf", [B_LOC, N, D], BF16),
        ("post_t", [B_LOC, D, N], BF16),
        ("wl_b", [D, D], BF16),
        ("wt_hp", [2 * D, K], BF16),
        ("wt_hr", [2 * D, K], BF16),
        ("whp_c", [K, 1], BF16),
        ("whr_c", [K, 1], BF16),
        ("ident", [128, 128], F32),
    ]:
        ins[name] = nc.declare_dram_parameter(name, shape, dt, isOutput=False)
    out_e = nc.declare_dram_parameter("out", [B_LOC, 2 * D], F32, isOutput=True)

    with tile.TileContext(nc) as tc:
        _body(nc, tc, ins, out_e)

    nc.compile()
    return nc


def _body(nc, tc, ins, out_e):
    from contextlib import ExitStack

    ctx = ExitStack()
    cpool = ctx.enter_context(tc.tile_pool(name="const", bufs=1))
    inpool = ctx.enter_context(tc.tile_pool(name="inputs", bufs=1))
    wk = ctx.enter_context(tc.tile_pool(name="work", bufs=2))
    dpool = ctx.enter_context(tc.tile_pool(name="dram", bufs=2, space="DRAM"))
    ps_mm = ctx.enter_context(tc.tile_pool(name="ps_mm", bufs=4, space="PSUM"))
    ps_acc = ctx.enter_context(tc.tile_pool(name="ps_acc", bufs=4, space="PSUM"))

    # ---------------- constants (all pre-transposed on host) ----------------
    ident_f = cpool.tile([128, 128], F32)
    nc.sync.dma_start(out=ident_f[:], in_=ins["ident"].ap())
    wl_b = cpool.tile([D, D], BF16)
    nc.sync.dma_start(out=wl_b[:], in_=ins["wl_b"].ap())
    wt_hp = cpool.tile([2 * D, K], BF16)
    nc.sync.dma_start(out=wt_hp[:], in_=ins["wt_hp"].ap())
    wt_hr = cpool.tile([2 * D, K], BF16)
    nc.sync.dma_start(out=wt_hr[:], in_=ins["wt_hr"].ap())
    whp_b = cpool.tile([K, 1], BF16)
    nc.sync.dma_start(out=whp_b[:], in_=ins["whp_c"].ap())
    whr_b = cpool.tile([K, 1], BF16)
    nc.sync.dma_start(out=whr_b[:], in_=ins["whr_c"].ap())

    # Persistent bf16 inputs (matmul operands + pooling-phase lhsT)
    r_ball = inpool.tile([128, B_LOC, TI, D], BF16)
    p_ball = inpool.tile([128, B_LOC, NI, D], BF16)

    # Per-batch logits, transposed layout: cols 0:4 ap n-tiles, 4:12 ar t-tiles
    lgt_all = inpool.tile([128, 12, B_LOC], F32)

    # ---------------- per-batch main phase ----------------
    # Two batches in flight: batch b's tail (Y, Hp, Hr, logits) is emitted
    # interleaved with batch b+1's head (loads, RlT, L, X) so every
    # cross-engine latency (tanh evacuations, the L->DRAM->xbar-transpose
    # round trip) is covered by independent PE work from the other batch.
    def make_batch(b):
        st = {}
        head = []
        tail = []

        def c_load():
            nc.sync.dma_start(
                out=r_ball[:, b],
                in_=ins["review_bf"].ap()[b].rearrange("(i p) d -> p i d", p=128),
            )
            nc.sync.dma_start(
                out=p_ball[:, b],
                in_=ins["post_bf"].ap()[b].rearrange("(j p) d -> p j d", p=128),
            )
            st["hr_in"] = wk.tile([128, T], BF16, tag="hr_in", name=f"hr_in{b}")
            nc.sync.dma_start(out=st["hr_in"][0:D, :], in_=ins["review_t"].ap()[b])
            st["hp_in"] = wk.tile([128, N], BF16, tag="hp_in", name=f"hp_in{b}")
            nc.sync.dma_start(out=st["hp_in"][0:D, :], in_=ins["post_t"].ap()[b])
            st["rlt"] = wk.tile([D, T], BF16, tag="rlt", name=f"rlt{b}")
            st["l_sb"] = wk.tile([128, TI, N], BF16, tag="l_sb", name=f"l_sb{b}")
            st["lt_sb"] = wk.tile([128, NI, T], BF16, tag="lt_sb", name=f"lt_sb{b}")
            st["l_dram"] = dpool.tile([T, N], BF16, tag="l_dram", name=f"l_dram{b}")
            st["lps"] = {}

        head.append(c_load)

        def c_rlt(c):
            rlt_ps = ps_mm.tile([D, 512], F32, tag="mm", name=f"rlt_ps{b}_{c}")
            nc.tensor.matmul(
                rlt_ps[:], wl_b[:], st["hr_in"][0:D, c * 512:(c + 1) * 512]
            )
            nc.vector.tensor_copy(st["rlt"][:, c * 512:(c + 1) * 512], rlt_ps[:])

        head.append(lambda: c_rlt(0))
        head.append(lambda: c_rlt(1))

        def emit_l_mm(i):
            st["lps"][i] = ps_mm.tile([128, N], F32, tag="mm", name=f"lps{b}_{i}")
            nc.tensor.matmul(
                st["lps"][i][:], st["rlt"][:, i * 128:(i + 1) * 128],
                st["hp_in"][0:D, :],
            )

        def emit_l_evac(i):
            nc.scalar.activation(st["l_sb"][:, i], st["lps"][i][:], AF.Tanh)
            l_dram_v = st["l_dram"].rearrange("(i p) n -> p i n", p=128)
            nc.sync.dma_start(out=l_dram_v[:, i], in_=st["l_sb"][:, i])
            if i == 3:
                for j in range(NI):
                    nc.sync.dma_start_transpose(
                        out=st["lt_sb"][:, j, 0:512],
                        in_=st["l_dram"][0:512, j * 128:(j + 1) * 128],
                    )
            elif i == TI - 1:
                for j in range(NI):
                    nc.sync.dma_start_transpose(
                        out=st["lt_sb"][:, j, 512:1024],
                        in_=st["l_dram"][512:1024, j * 128:(j + 1) * 128],
                    )

        def c_l_start():
            st["xps"] = ps_acc.tile([D, N], F32, tag="acc", name=f"xps{b}")
            emit_l_mm(0)
            emit_l_mm(1)
            emit_l_mm(2)
            emit_l_evac(0)

        head.append(c_l_start)

        def c_x(i):
            nc.tensor.matmul(
                st["xps"][:], r_ball[:, b, i], st["l_sb"][:, i],
                start=(i == 0), stop=(i == TI - 1),
            )
            if i + 3 < TI:
                emit_l_mm(i + 3)
            if i + 1 < TI:
                emit_l_evac(i + 1)
            if i == TI - 1:
                emit_l_evac(i)
                nc.vector.tensor_copy(st["hp_in"][D:128, :], st["xps"][:])

        for i in range(TI):
            head.append(lambda i=i: c_x(i))

        def t_y0():
            st["yps"] = [
                ps_acc.tile([D, 512], F32, tag="acc", name=f"yps{b}_{c}")
                for c in range(2)
            ]
            for j in range(NI):
                nc.tensor.matmul(
                    st["yps"][0][:], p_ball[:, b, j], st["lt_sb"][:, j, 0:512],
                    start=(j == 0), stop=(j == NI - 1),
                )
            nc.vector.tensor_copy(st["hr_in"][D:128, 0:512], st["yps"][0][:])

        tail.append(t_y0)

        def t_hp():
            hp_bf = wk.tile([K, N], BF16, tag="hp_bf", name=f"hp_bf{b}")
            st["hp_bf"] = hp_bf
            hps = ps_acc.tile([K, N], F32, tag="acc", name=f"hps{b}")
            nc.tensor.matmul(hps[:], wt_hp[:], st["hp_in"][:])
            nc.scalar.activation(hp_bf[:], hps[:], AF.Tanh)

        tail.append(t_hp)

        def t_y1():
            for j in range(NI):
                nc.tensor.matmul(
                    st["yps"][1][:], p_ball[:, b, j], st["lt_sb"][:, j, 512:1024],
                    start=(j == 0), stop=(j == NI - 1),
                )
            nc.vector.tensor_copy(st["hr_in"][D:128, 512:1024], st["yps"][1][:])

        tail.append(t_y1)

        def t_hr(c):
            if c == 0:
                st["hr_bf"] = wk.tile([K, T], BF16, tag="hr_bf", name=f"hr_bf{b}")
            hrs = ps_acc.tile([K, 512], F32, tag="acc", name=f"hrs{b}_{c}")
            nc.tensor.matmul(hrs[:], wt_hr[:], st["hr_in"][:, c * 512:(c + 1) * 512])
            nc.scalar.activation(
                st["hr_bf"][:, c * 512:(c + 1) * 512], hrs[:], AF.Tanh
            )

        tail.append(lambda: t_hr(0))
        tail.append(lambda: t_hr(1))

        def t_logits():
            lg_ps = ps_acc.tile([128, 12], F32, tag="acc", name=f"lg_ps{b}")
            for j in range(NI):
                nc.tensor.matmul(
                    lg_ps[:, j:j + 1], st["hp_bf"][:, j * 128:(j + 1) * 128],
                    whp_b[:], skip_group_check=True,
                )
            for i in range(TI):
                nc.tensor.matmul(
                    lg_ps[:, 4 + i:5 + i], st["hr_bf"][:, i * 128:(i + 1) * 128],
                    whr_b[:], skip_group_check=True,
                )
            nc.vector.tensor_copy(lgt_all[:, :, b], lg_ps[:])

        tail.append(t_logits)

        return head, tail

    prev_tail = []
    for b in range(B_LOC):
        head, tail = make_batch(b)
        # merge: head chunks of b interleaved with tail chunks of b-1
        n = max(len(head), len(prev_tail))
        hi = ti = 0
        for s in range(n):
            if hi < len(head):
                head[hi]()
                hi += 1
            if ti < len(prev_tail):
                prev_tail[ti]()
                ti += 1
        while hi < len(head):
            head[hi]()
            hi += 1
        prev_tail = tail
    for t in prev_tail:
        t()

    # ---------------- softmax phase (all batches on partitions) ----------------
    logits = inpool.tile([B_LOC, 12 * 128], F32)
    for g in range(3):
        lgt_t_ps = ps_acc.tile([B_LOC, 512], F32, tag="acc")
        for jj in range(4):
            j = g * 4 + jj
            nc.tensor.transpose(
                lgt_t_ps[:, jj * 128:(jj + 1) * 128], lgt_all[:, j, :], ident_f[:]
            )
        nc.vector.tensor_copy(logits[:, g * 512:(g + 1) * 512], lgt_t_ps[:])

    mx = inpool.tile([B_LOC, 2], F32)
    nc.vector.reduce_max(mx[:, 0:1], logits[:, 0:N], axis=mybir.AxisListType.X)
    nc.vector.reduce_max(mx[:, 1:2], logits[:, N:N + T], axis=mybir.AxisListType.X)
    nmx = inpool.tile([B_LOC, 2], F32)
    nc.vector.tensor_scalar_mul(nmx[:], mx[:], -1.0)

    probs = inpool.tile([B_LOC, 12 * 128], F32)
    sums = inpool.tile([B_LOC, 2], F32)
    nc.scalar.activation(
        probs[:, 0:N], logits[:, 0:N], AF.Exp, bias=nmx[:, 0:1], accum_out=sums[:, 0:1]
    )
    nc.scalar.activation(
        probs[:, N:N + T], logits[:, N:N + T], AF.Exp, bias=nmx[:, 1:2],
        accum_out=sums[:, 1:2],
    )
    rcp = inpool.tile([B_LOC, 2], F32)
    nc.vector.reciprocal(rcp[:], sums[:])
    pn = inpool.tile([B_LOC, 12 * 128], F32)
    nc.vector.tensor_scalar_mul(pn[:, 0:N], probs[:, 0:N], rcp[:, 0:1])
    nc.vector.tensor_scalar_mul(pn[:, N:N + T], probs[:, N:N + T], rcp[:, 1:2])

    # Transpose probs back to partition-major bf16: PrT[:, j, b]
    prt = inpool.tile([128, 12, B_LOC], BF16)
    prt_ps = ps_acc.tile([128, 12 * B_LOC], F32, tag="acc")
    for j in range(12):
        nc.tensor.transpose(
            prt_ps[:, j * B_LOC:(j + 1) * B_LOC],
            pn[:, j * 128:(j + 1) * 128],
            ident_f[0:B_LOC, 0:B_LOC],
        )
    nc.vector.tensor_copy(prt[:], prt_ps[:])

    # ---------------- pooling phase ----------------
    # co_all (64, 16): col b = P_b^T @ Ap_b, col 8+b = R_b^T @ Ar_b
    co_ps = ps_acc.tile([D, 2 * B_LOC], F32, tag="acc")
    for b in range(B_LOC):
        for j in range(NI):
            nc.tensor.matmul(
                co_ps[:, b:b + 1], p_ball[:, b, j], prt[:, j, b:b + 1],
                start=(j == 0), stop=(j == NI - 1), skip_group_check=True,
            )
        for i in range(TI):
            nc.tensor.matmul(
                co_ps[:, B_LOC + b:B_LOC + b + 1], r_ball[:, b, i],
                prt[:, 4 + i, b:b + 1],
                start=(i == 0), stop=(i == TI - 1), skip_group_check=True,
            )
    co_sb = inpool.tile([D, 2 * B_LOC], F32)
    nc.vector.tensor_copy(co_sb[:], co_ps[:])

    # Transpose (64, 16) -> (16, 64); row h*8+b is the h-half of out[b]
    cot_ps = ps_acc.tile([2 * B_LOC, D], F32, tag="acc")
    nc.tensor.transpose(cot_ps[:], co_sb[:], ident_f[0:D, 0:D])
    out_sb = inpool.tile([2 * B_LOC, D], F32)
    nc.vector.tensor_copy(out_sb[:], cot_ps[:])
    nc.sync.dma_start(out=out_e.ap()[:, 0:D], in_=out_sb[0:B_LOC, :])
    nc.sync.dma_start(out=out_e.ap()[:, D:2 * D], in_=out_sb[B_LOC:2 * B_LOC, :])
    ctx.close()


_NC_CACHE = None


def _get_nc():
    global _NC_CACHE
    if _NC_CACHE is None:
        _NC_CACHE = build_kernel()
    return _NC_CACHE


def _prep_host_inputs(inputs):
    import ml_dtypes

    bf = ml_dtypes.bfloat16
    rev = np.ascontiguousarray(np.asarray(inputs["review_seq"], dtype=np.float32))
    post = np.ascontiguousarray(np.asarray(inputs["post_seq"], dtype=np.float32))
    wl = np.asarray(inputs["Wl"], dtype=np.float32)
    wr = np.asarray(inputs["Wr"], dtype=np.float32)
    wp = np.asarray(inputs["Wp"], dtype=np.float32)
    whr = np.asarray(inputs["whr"], dtype=np.float32)
    whp = np.asarray(inputs["whp"], dtype=np.float32)

    rev_bf = rev.astype(bf)
    post_bf = post.astype(bf)
    rev_t = np.ascontiguousarray(np.swapaxes(rev_bf, 1, 2))
    post_t = np.ascontiguousarray(np.swapaxes(post_bf, 1, 2))
    const = {
        "wl_b": np.ascontiguousarray(wl.astype(bf)),
        "wt_hp": np.ascontiguousarray(np.concatenate([wp.T, wr.T], axis=0).astype(bf)),
        "wt_hr": np.ascontiguousarray(np.concatenate([wr.T, wp.T], axis=0).astype(bf)),
        "whp_c": np.ascontiguousarray(whp.T.astype(bf)),
        "whr_c": np.ascontiguousarray(whr.T.astype(bf)),
        "ident": np.eye(128, dtype=np.float32),
    }
    return rev_bf, rev_t, post_bf, post_t, const


def run_on_hw(inputs: dict, trace: bool = False, **kw):
    nc = _get_nc()
    rev_bf, rev_t, post_bf, post_t, const = _prep_host_inputs(inputs)
    in_maps = []
    for c in range(NCORES):
        s = slice(c * B_LOC, (c + 1) * B_LOC)
        m = {
            "review_bf": np.ascontiguousarray(rev_bf[s]),
            "review_t": np.ascontiguousarray(rev_t[s]),
            "post_bf": np.ascontiguousarray(post_bf[s]),
            "post_t": np.ascontiguousarray(post_t[s]),
        }
        m.update(const)
        in_maps.append(m)
    res = run_bass_kernel_spmd(nc, in_maps, list(range(NCORES)), trace=trace, **kw)
    out = np.concatenate([res.results[c]["out"] for c in range(NCORES)], axis=0)
    return out, res


def kernel(**inputs) -> np.ndarray:
    out, _ = run_on_hw(inputs, trace=False)
    return out.astype(np.float32)


# revision 17
# speedup vs baseline: 1.2465x; 1.2465x over previous
"""CoAttLayer Trainium2 kernel — pure data-parallel over batch on 8 NeuronCores.

Reference computation (per batch element b, T=1024, N=512, D=64, K=80):
  L  = tanh(R @ Wl @ P^T)                    (T, N)
  Hp = tanh(Wp @ P^T + (Wr @ R^T) @ L)       (K, N)
  Hr = tanh(Wr @ R^T + (Wp @ P^T) @ L^T)     (K, T)
  Ap = softmax(whp @ Hp), Ar = softmax(whr @ Hr)
  out[b] = concat(P^T @ Ap, R^T @ Ar)        (2D,)

Reassociated into D-sized contractions:
  Hp = [Wp | Wr] @ [P^T ; X]   with X = R^T @ L    (D, N)
  Hr = [Wr | Wp] @ [R^T ; Y]   with Y = P^T @ L^T  (D, T)

Design notes (from trace analysis):
 - The PE HAM clock governor only counts real matmul activity; transpose-mode
   instructions poison it back to 1.2 GHz. So the batch loop contains ZERO PE
   transposes: all static transposed layouts (R^T, P^T, weight stacks) are
   prepared on the HOST, and the data-dependent L^T is produced by bouncing
   L through DRAM and reading it back through the DMA xbar transpose engine
   (~180 GB/s, fully off the compute engines).
 - All matmul operands are bf16 (fp32 PSUM accumulate); tanh lives on the
   Scalar engine with 1024-wide evacuations; PSUM evacuations go to DVE.
 - Softmax is batched across the 8 local batch elements on partitions.
"""

import numpy as np

import concourse.bass as bass
import concourse.bacc as bacc
import concourse.mybir as mybir
import concourse.tile as tile
from concourse.bass_utils import run_bass_kernel_spmd

F32 = mybir.dt.float32
BF16 = mybir.dt.bfloat16
AF = mybir.ActivationFunctionType

B_LOC = 8      # batch elements per core
T, N, D, K = 1024, 512, 64, 80
TI = T // 128  # 8 t-tiles
NI = N // 128  # 4 n-tiles
NCORES = 8


def build_kernel():
    nc = bacc.Bacc("TRN2", debug=False, target_bir_lowering=False)

    ins = {}
    for name, shape, dt in [
        ("review_bf", [B_LOC, T, D], BF16),
        ("review_t", [B_LOC, D, T], BF16),
        ("post_bf", [B_LOC, N, D], BF16),
        ("post_t", [B_LOC, D, N], BF16),
        ("wl2", [2 * D, D], BF16),
        ("wt_hp", [2 * D, K], BF16),
        ("wt_hr", [2 * D, K], BF16),
        ("whp_c", [K, 1], BF16),
        ("whr_c", [K, 1], BF16),
        ("ident", [128, 128], F32),
    ]:
        ins[name] = nc.declare_dram_parameter(name, shape, dt, isOutput=False)
    out_e = nc.declare_dram_parameter("out", [B_LOC, 2 * D], F32, isOutput=True)

    with tile.TileContext(nc) as tc:
        _body(nc, tc, ins, out_e)

    nc.compile()
    return nc


def _body(nc, tc, ins, out_e):
    from contextlib import ExitStack

    ctx = ExitStack()
    cpool = ctx.enter_context(tc.tile_pool(name="const", bufs=1))
    inpool = ctx.enter_context(tc.tile_pool(name="inputs", bufs=1))
    wk = ctx.enter_context(tc.tile_pool(name="work", bufs=2))
    dpool = ctx.enter_context(tc.tile_pool(name="dram", bufs=2, space="DRAM"))
    ps_mm = ctx.enter_context(tc.tile_pool(name="ps_mm", bufs=2, space="PSUM"))
    ps_acc = ctx.enter_context(tc.tile_pool(name="ps_acc", bufs=4, space="PSUM"))

    # ---------------- constants (all pre-transposed on host) ----------------
    ident_f = cpool.tile([128, 128], F32)
    nc.sync.dma_start(out=ident_f[:], in_=ins["ident"].ap())
    wl2 = cpool.tile([2 * D, D], BF16)
    nc.sync.dma_start(out=wl2[:], in_=ins["wl2"].ap())
    wt_hp = cpool.tile([2 * D, K], BF16)
    nc.sync.dma_start(out=wt_hp[:], in_=ins["wt_hp"].ap())
    wt_hr = cpool.tile([2 * D, K], BF16)
    nc.sync.dma_start(out=wt_hr[:], in_=ins["wt_hr"].ap())
    whp_b = cpool.tile([K, 1], BF16)
    nc.sync.dma_start(out=whp_b[:], in_=ins["whp_c"].ap())
    whr_b = cpool.tile([K, 1], BF16)
    nc.sync.dma_start(out=whr_b[:], in_=ins["whr_c"].ap())

    # Persistent bf16 inputs, one tile per batch (avoids false whole-tile deps)
    rbp = ctx.enter_context(tc.tile_pool(name="rbp", bufs=2 * B_LOC))
    r_b = [rbp.tile([128, TI, D], BF16, tag="r", name=f"r_b{b}") for b in range(B_LOC)]
    p_b = [rbp.tile([128, NI, D], BF16, tag="p", name=f"p_b{b}") for b in range(B_LOC)]

    # Per-batch logits, transposed layout: cols 0:4 ap n-tiles, 4:12 ar t-tiles
    lgt_all = inpool.tile([128, 12, B_LOC], F32)

    # ---------------- per-batch main phase ----------------
    # Two batches in flight; K=64 matmuls (RlT, L) are packed two-at-a-time
    # into disjoint PE row-groups via tile_position (K<=64 streams at half
    # rate unpacked — measured 427 vs 117 ns per matmul for N=512).
    def make_batch(b):
        st = {}
        head = []
        tail = []

        def c_load():
            nc.sync.dma_start(
                out=r_b[b][:],
                in_=ins["review_bf"].ap()[b].rearrange("(i p) d -> p i d", p=128),
            )
            nc.sync.dma_start(
                out=p_b[b][:],
                in_=ins["post_bf"].ap()[b].rearrange("(j p) d -> p j d", p=128),
            )
            st["hr_in"] = wk.tile([128, T], BF16, tag="hr_in", name=f"hr_in{b}")
            nc.sync.dma_start(out=st["hr_in"][0:D, :], in_=ins["review_t"].ap()[b])
            st["hp_in"] = wk.tile([128, N], BF16, tag="hp_in", name=f"hp_in{b}")
            nc.sync.dma_start(out=st["hp_in"][0:D, :], in_=ins["post_t"].ap()[b])
            # replicated layouts for row-packed K=64 matmuls
            st["rt2"] = wk.tile([128, T], BF16, tag="rt2", name=f"rt2{b}")
            nc.sync.dma_start(out=st["rt2"][0:D, :], in_=ins["review_t"].ap()[b])
            nc.sync.dma_start(out=st["rt2"][D:128, :], in_=ins["review_t"].ap()[b])
            st["pt2"] = wk.tile([128, N], BF16, tag="pt2", name=f"pt2{b}")
            nc.sync.dma_start(out=st["pt2"][0:D, :], in_=ins["post_t"].ap()[b])
            nc.sync.dma_start(out=st["pt2"][D:128, :], in_=ins["post_t"].ap()[b])
            st["rlt2"] = wk.tile([128, N], BF16, tag="rlt2", name=f"rlt2{b}")
            st["l_sb"] = wk.tile([128, TI, N], BF16, tag="l_sb", name=f"l_sb{b}")
            st["lt_sb"] = wk.tile([128, NI, T], BF16, tag="lt_sb", name=f"lt_sb{b}")
            st["l_dram"] = dpool.tile([T, N], BF16, tag="l_dram", name=f"l_dram{b}")
            st["lps"] = {}

        head.append(c_load)

        def c_rlt():
            # rlt2 layout: top half = RlT chunks 0,2,4,6; bottom = 1,3,5,7.
            # Each half computed by one member of a row-packed matmul pair
            # whose rhs is the even/odd-interleaved view of (replicated) Rt.
            rt_v = [
                st["rt2"][h * D:(h + 1) * D, :]
                .rearrange("p (c two k) -> p two c k", two=2, k=128)[:, h]
                for h in range(2)
            ]
            pss = []
            for h in range(2):
                ps = ps_mm.tile([D, 512], F32, tag="mm", name=f"rlt_ps{b}_{h}")
                nc.tensor.matmul(
                    ps[:], wl2[h * D:(h + 1) * D, :], rt_v[h],
                    tile_position=(h * D, 0),
                )
                pss.append(ps)
            for h in range(2):
                nc.vector.tensor_copy(st["rlt2"][h * D:(h + 1) * D, :], pss[h][:])

        head.append(c_rlt)

        def emit_l_pair(p):
            lp = ps_mm.tile([128, 2, N], F32, tag="mm", name=f"lps{b}_{p}")
            st["lps"][p] = lp
            for h in range(2):
                nc.tensor.matmul(
                    lp[:, h],
                    st["rlt2"][h * D:(h + 1) * D, p * 128:(p + 1) * 128],
                    st["pt2"][h * D:(h + 1) * D, :],
                    tile_position=(h * D, 0),
                )

        def emit_l_evac(p):
            nc.scalar.activation(
                st["l_sb"][:, 2 * p:2 * p + 2, :], st["lps"][p][:], AF.Tanh
            )
            l_dram_v = st["l_dram"].rearrange("(i p) n -> p i n", p=128)
            nc.sync.dma_start(
                out=l_dram_v[:, 2 * p:2 * p + 2, :],
                in_=st["l_sb"][:, 2 * p:2 * p + 2, :],
            )
            if p == 1:
                for j in range(NI):
                    nc.sync.dma_start_transpose(
                        out=st["lt_sb"][:, j, 0:512],
                        in_=st["l_dram"][0:512, j * 128:(j + 1) * 128],
                    )
            elif p == 3:
                for j in range(NI):
                    nc.sync.dma_start_transpose(
                        out=st["lt_sb"][:, j, 512:1024],
                        in_=st["l_dram"][512:1024, j * 128:(j + 1) * 128],
                    )

        def c_l_start():
            st["xps"] = ps_acc.tile([D, N], F32, tag="acc", name=f"xps{b}")
            emit_l_pair(0)
            emit_l_pair(1)
            emit_l_evac(0)

        head.append(c_l_start)

        def c_x(p):
            for i in (2 * p, 2 * p + 1):
                nc.tensor.matmul(
                    st["xps"][:], r_b[b][:, i], st["l_sb"][:, i],
                    start=(i == 0), stop=(i == TI - 1),
                )
            if p + 2 < TI // 2:
                emit_l_pair(p + 2)
            if p + 1 < TI // 2:
                emit_l_evac(p + 1)
            if p == TI // 2 - 1:
                nc.vector.tensor_copy(st["hp_in"][D:128, :], st["xps"][:])

        for p in range(TI // 2):
            head.append(lambda p=p: c_x(p))

        def t_y0():
            st["yps"] = [
                ps_acc.tile([D, 512], F32, tag="acc", name=f"yps{b}_{c}")
                for c in range(2)
            ]
            for j in range(NI):
                nc.tensor.matmul(
                    st["yps"][0][:], p_b[b][:, j], st["lt_sb"][:, j, 0:512],
                    start=(j == 0), stop=(j == NI - 1),
                )
            nc.vector.tensor_copy(st["hr_in"][D:128, 0:512], st["yps"][0][:])

        tail.append(t_y0)

        def t_hp():
            hp_bf = wk.tile([K, N], BF16, tag="hp_bf", name=f"hp_bf{b}")
            st["hp_bf"] = hp_bf
            hps = ps_acc.tile([K, N], F32, tag="acc", name=f"hps{b}")
            nc.tensor.matmul(hps[:], wt_hp[:], st["hp_in"][:])
            nc.scalar.activation(hp_bf[:], hps[:], AF.Tanh)

        tail.append(t_hp)

        def t_y1():
            for j in range(NI):
                nc.tensor.matmul(
                    st["yps"][1][:], p_b[b][:, j], st["lt_sb"][:, j, 512:1024],
                    start=(j == 0), stop=(j == NI - 1),
                )
            nc.vector.tensor_copy(st["hr_in"][D:128, 512:1024], st["yps"][1][:])

        tail.append(t_y1)

        def t_hr(c):
            if c == 0:
                st["hr_bf"] = wk.tile([K, T], BF16, tag="hr_bf", name=f"hr_bf{b}")
            hrs = ps_acc.tile([K, 512], F32, tag="acc", name=f"hrs{b}_{c}")
            nc.tensor.matmul(hrs[:], wt_hr[:], st["hr_in"][:, c * 512:(c + 1) * 512])
            nc.scalar.activation(
                st["hr_bf"][:, c * 512:(c + 1) * 512], hrs[:], AF.Tanh
            )

        tail.append(lambda: t_hr(0))
        tail.append(lambda: t_hr(1))

        def t_logits():
            lg_ps = ps_acc.tile([128, 12], F32, tag="acc", name=f"lg_ps{b}")
            for j in range(NI):
                nc.tensor.matmul(
                    lg_ps[:, j:j + 1], st["hp_bf"][:, j * 128:(j + 1) * 128],
                    whp_b[:], skip_group_check=True,
                )
            for i in range(TI):
                nc.tensor.matmul(
                    lg_ps[:, 4 + i:5 + i], st["hr_bf"][:, i * 128:(i + 1) * 128],
                    whr_b[:], skip_group_check=True,
                )
            nc.vector.tensor_copy(lgt_all[:, :, b], lg_ps[:])

        tail.append(t_logits)

        return head, tail

    prev_tail = []
    for b in range(B_LOC):
        head, tail = make_batch(b)
        hi = ti = 0
        for s in range(max(len(head), len(prev_tail))):
            if hi < len(head):
                head[hi]()
                hi += 1
            if ti < len(prev_tail):
                prev_tail[ti]()
                ti += 1
        prev_tail = tail
    for t in prev_tail:
        t()

    # ---------------- softmax phase (all batches on partitions) ----------------
    logits = inpool.tile([B_LOC, 12 * 128], F32)
    for g in range(3):
        lgt_t_ps = ps_acc.tile([B_LOC, 512], F32, tag="acc")
        for jj in range(4):
            j = g * 4 + jj
            nc.tensor.transpose(
                lgt_t_ps[:, jj * 128:(jj + 1) * 128], lgt_all[:, j, :], ident_f[:]
            )
        nc.vector.tensor_copy(logits[:, g * 512:(g + 1) * 512], lgt_t_ps[:])

    mx = inpool.tile([B_LOC, 2], F32)
    nc.vector.reduce_max(mx[:, 0:1], logits[:, 0:N], axis=mybir.AxisListType.X)
    nc.vector.reduce_max(mx[:, 1:2], logits[:, N:N + T], axis=mybir.AxisListType.X)
    nmx = inpool.tile([B_LOC, 2], F32)
    nc.vector.tensor_scalar_mul(nmx[:], mx[:], -1.0)

    probs = inpool.tile([B_LOC, 12 * 128], F32)
    sums = inpool.tile([B_LOC, 2], F32)
    nc.scalar.activation(
        probs[:, 0:N], logits[:, 0:N], AF.Exp, bias=nmx[:, 0:1], accum_out=sums[:, 0:1]
    )
    nc.scalar.activation(
        probs[:, N:N + T], logits[:, N:N + T], AF.Exp, bias=nmx[:, 1:2],
        accum_out=sums[:, 1:2],
    )
    rcp = inpool.tile([B_LOC, 2], F32)
    nc.vector.reciprocal(rcp[:], sums[:])
    pn = inpool.tile([B_LOC, 12 * 128], F32)
    nc.vector.tensor_scalar_mul(pn[:, 0:N], probs[:, 0:N], rcp[:, 0:1])
    nc.vector.tensor_scalar_mul(pn[:, N:N + T], probs[:, N:N + T], rcp[:, 1:2])

    # Transpose probs back to partition-major bf16: PrT[:, j, b]
    prt = inpool.tile([128, 12, B_LOC], BF16)
    prt_ps = ps_acc.tile([128, 12 * B_LOC], F32, tag="acc")
    for j in range(12):
        nc.tensor.transpose(
            prt_ps[:, j * B_LOC:(j + 1) * B_LOC],
            pn[:, j * 128:(j + 1) * 128],
            ident_f[0:B_LOC, 0:B_LOC],
        )
    nc.vector.tensor_copy(prt[:], prt_ps[:])

    # ---------------- pooling phase ----------------
    # co_all (64, 16): col b = P_b^T @ Ap_b, col 8+b = R_b^T @ Ar_b
    co_ps = ps_acc.tile([D, 2 * B_LOC], F32, tag="acc")
    for b in range(B_LOC):
        for j in range(NI):
            nc.tensor.matmul(
                co_ps[:, b:b + 1], p_b[b][:, j], prt[:, j, b:b + 1],
                start=(j == 0), stop=(j == NI - 1), skip_group_check=True,
            )
        for i in range(TI):
            nc.tensor.matmul(
                co_ps[:, B_LOC + b:B_LOC + b + 1], r_b[b][:, i],
                prt[:, 4 + i, b:b + 1],
                start=(i == 0), stop=(i == TI - 1), skip_group_check=True,
            )
    co_sb = inpool.tile([D, 2 * B_LOC], F32)
    nc.vector.tensor_copy(co_sb[:], co_ps[:])

    # Transpose (64, 16) -> (16, 64); row h*8+b is the h-half of out[b]
    cot_ps = ps_acc.tile([2 * B_LOC, D], F32, tag="acc")
    nc.tensor.transpose(cot_ps[:], co_sb[:], ident_f[0:D, 0:D])
    out_sb = inpool.tile([2 * B_LOC, D], F32)
    nc.vector.tensor_copy(out_sb[:], cot_ps[:])
    nc.sync.dma_start(out=out_e.ap()[:, 0:D], in_=out_sb[0:B_LOC, :])
    nc.sync.dma_start(out=out_e.ap()[:, D:2 * D], in_=out_sb[B_LOC:2 * B_LOC, :])
    ctx.close()


_NC_CACHE = None


def _get_nc():
    global _NC_CACHE
    if _NC_CACHE is None:
        _NC_CACHE = build_kernel()
    return _NC_CACHE


def _prep_host_inputs(inputs):
    import ml_dtypes

    bf = ml_dtypes.bfloat16
    rev = np.ascontiguousarray(np.asarray(inputs["review_seq"], dtype=np.float32))
    post = np.ascontiguousarray(np.asarray(inputs["post_seq"], dtype=np.float32))
    wl = np.asarray(inputs["Wl"], dtype=np.float32)
    wr = np.asarray(inputs["Wr"], dtype=np.float32)
    wp = np.asarray(inputs["Wp"], dtype=np.float32)
    whr = np.asarray(inputs["whr"], dtype=np.float32)
    whp = np.asarray(inputs["whp"], dtype=np.float32)

    rev_bf = rev.astype(bf)
    post_bf = post.astype(bf)
    rev_t = np.ascontiguousarray(np.swapaxes(rev_bf, 1, 2))
    post_t = np.ascontiguousarray(np.swapaxes(post_bf, 1, 2))
    const = {
        "wl2": np.ascontiguousarray(np.concatenate([wl, wl], axis=0).astype(bf)),
        "wt_hp": np.ascontiguousarray(np.concatenate([wp.T, wr.T], axis=0).astype(bf)),
        "wt_hr": np.ascontiguousarray(np.concatenate([wr.T, wp.T], axis=0).astype(bf)),
        "whp_c": np.ascontiguousarray(whp.T.astype(bf)),
        "whr_c": np.ascontiguousarray(whr.T.astype(bf)),
        "ident": np.eye(128, dtype=np.float32),
    }
    return rev_bf, rev_t, post_bf, post_t, const


def run_on_hw(inputs: dict, trace: bool = False, **kw):
    nc = _get_nc()
    rev_bf, rev_t, post_bf, post_t, const = _prep_host_inputs(inputs)
    in_maps = []
    for c in range(NCORES):
        s = slice(c * B_LOC, (c + 1) * B_LOC)
        m = {
            "review_bf": np.ascontiguousarray(rev_bf[s]),
            "review_t": np.ascontiguousarray(rev_t[s]),
            "post_bf": np.ascontiguousarray(post_bf[s]),
            "post_t": np.ascontiguousarray(post_t[s]),
        }
        m.update(const)
        in_maps.append(m)
    res = run_bass_kernel_spmd(nc, in_maps, list(range(NCORES)), trace=trace, **kw)
    out = np.concatenate([res.results[c]["out"] for c in range(NCORES)], axis=0)
    return out, res


def kernel(**inputs) -> np.ndarray:
    out, _ = run_on_hw(inputs, trace=False)
    return out.astype(np.float32)


# revision 18
# speedup vs baseline: 1.2619x; 1.0123x over previous
"""CoAttLayer Trainium2 kernel — pure data-parallel over batch on 8 NeuronCores.

Reference computation (per batch element b, T=1024, N=512, D=64, K=80):
  L  = tanh(R @ Wl @ P^T)                    (T, N)
  Hp = tanh(Wp @ P^T + (Wr @ R^T) @ L)       (K, N)
  Hr = tanh(Wr @ R^T + (Wp @ P^T) @ L^T)     (K, T)
  Ap = softmax(whp @ Hp), Ar = softmax(whr @ Hr)
  out[b] = concat(P^T @ Ap, R^T @ Ar)        (2D,)

Reassociated into D-sized contractions:
  Hp = [Wp | Wr] @ [P^T ; X]   with X = R^T @ L    (D, N)
  Hr = [Wr | Wp] @ [R^T ; Y]   with Y = P^T @ L^T  (D, T)

Design notes (from trace analysis):
 - The PE HAM clock governor only counts real matmul activity; transpose-mode
   instructions poison it back to 1.2 GHz. So the batch loop contains ZERO PE
   transposes: all static transposed layouts (R^T, P^T, weight stacks) are
   prepared on the HOST, and the data-dependent L^T is produced by bouncing
   L through DRAM and reading it back through the DMA xbar transpose engine
   (~180 GB/s, fully off the compute engines).
 - All matmul operands are bf16 (fp32 PSUM accumulate); tanh lives on the
   Scalar engine with 1024-wide evacuations; PSUM evacuations go to DVE.
 - Softmax is batched across the 8 local batch elements on partitions.
"""

import numpy as np

import concourse.bass as bass
import concourse.bacc as bacc
import concourse.mybir as mybir
import concourse.tile as tile
from concourse.bass_utils import run_bass_kernel_spmd

F32 = mybir.dt.float32
BF16 = mybir.dt.bfloat16
AF = mybir.ActivationFunctionType

B_LOC = 8      # batch elements per core
T, N, D, K = 1024, 512, 64, 80
TI = T // 128  # 8 t-tiles
NI = N // 128  # 4 n-tiles
NCORES = 8


def build_kernel():
    nc = bacc.Bacc("TRN2", debug=False, target_bir_lowering=False)

    ins = {}
    for name, shape, dt in [
        ("review_bf", [B_LOC, T, D], BF16),
        ("review_t", [B_LOC, D, T], BF16),
        ("post_bf", [B_LOC, N, D], BF16),
        ("post_t", [B_LOC, D, N], BF16),
        ("wl2", [2 * D, D], BF16),
        ("wt_hp", [2 * D, K], BF16),
        ("wt_hr", [2 * D, K], BF16),
        ("whp_c", [K, 1], BF16),
        ("whr_c", [K, 1], BF16),
        ("ident", [128, 128], F32),
    ]:
        ins[name] = nc.declare_dram_parameter(name, shape, dt, isOutput=False)
    out_e = nc.declare_dram_parameter("out", [B_LOC, 2 * D], F32, isOutput=True)

    with tile.TileContext(nc) as tc:
        _body(nc, tc, ins, out_e)

    nc.compile()
    return nc


def _body(nc, tc, ins, out_e):
    from contextlib import ExitStack

    ctx = ExitStack()
    cpool = ctx.enter_context(tc.tile_pool(name="const", bufs=1))
    inpool = ctx.enter_context(tc.tile_pool(name="inputs", bufs=1))
    wk = ctx.enter_context(tc.tile_pool(name="work", bufs=2))
    dpool = ctx.enter_context(tc.tile_pool(name="dram", bufs=2, space="DRAM"))
    ps_mm = ctx.enter_context(tc.tile_pool(name="ps_mm", bufs=2, space="PSUM"))
    ps_acc = ctx.enter_context(tc.tile_pool(name="ps_acc", bufs=4, space="PSUM"))

    # ---------------- constants (all pre-transposed on host) ----------------
    ident_f = cpool.tile([128, 128], F32)
    nc.sync.dma_start(out=ident_f[:], in_=ins["ident"].ap())
    wl2 = cpool.tile([2 * D, D], BF16)
    nc.sync.dma_start(out=wl2[:], in_=ins["wl2"].ap())
    wt_hp = cpool.tile([2 * D, K], BF16)
    nc.sync.dma_start(out=wt_hp[:], in_=ins["wt_hp"].ap())
    wt_hr = cpool.tile([2 * D, K], BF16)
    nc.sync.dma_start(out=wt_hr[:], in_=ins["wt_hr"].ap())
    whp_b = cpool.tile([K, 1], BF16)
    nc.sync.dma_start(out=whp_b[:], in_=ins["whp_c"].ap())
    whr_b = cpool.tile([K, 1], BF16)
    nc.sync.dma_start(out=whr_b[:], in_=ins["whr_c"].ap())

    # Persistent bf16 inputs, one tile per batch (avoids false whole-tile deps)
    rbp = ctx.enter_context(tc.tile_pool(name="rbp", bufs=2 * B_LOC))
    r_b = [rbp.tile([128, TI, D], BF16, tag="r", name=f"r_b{b}") for b in range(B_LOC)]
    p_b = [rbp.tile([128, NI, D], BF16, tag="p", name=f"p_b{b}") for b in range(B_LOC)]

    # Per-batch logits, transposed layout: cols 0:4 ap n-tiles, 4:12 ar t-tiles
    lgt_all = inpool.tile([128, 12, B_LOC], F32)

    # ---------------- per-batch main phase ----------------
    # Two batches in flight; K=64 matmuls (RlT, L) are packed two-at-a-time
    # into disjoint PE row-groups via tile_position (K<=64 streams at half
    # rate unpacked — measured 427 vs 117 ns per matmul for N=512).
    def make_batch(b):
        st = {}
        head = []
        tail = []

        def c_load():
            # t-index convention: t = 8p + i (contiguous per-partition loads);
            # review_t/post_t are host-prepared in the matching column order.
            nc.sync.dma_start(
                out=r_b[b][:],
                in_=ins["review_bf"].ap()[b].rearrange("(p i) d -> p i d", i=TI),
            )
            nc.sync.dma_start(
                out=p_b[b][:],
                in_=ins["post_bf"].ap()[b].rearrange("(p j) d -> p j d", j=NI),
            )
            st["hr_in"] = wk.tile([128, T], BF16, tag="hr_in", name=f"hr_in{b}")
            nc.sync.dma_start(out=st["hr_in"][0:D, :], in_=ins["review_t"].ap()[b])
            st["hp_in"] = wk.tile([128, N], BF16, tag="hp_in", name=f"hp_in{b}")
            nc.sync.dma_start(out=st["hp_in"][0:D, :], in_=ins["post_t"].ap()[b])
            # bottom-half replicas for the row-packed K=64 matmuls (SBUF->SBUF)
            st["rt2"] = wk.tile([128, T], BF16, tag="rt2", name=f"rt2{b}")
            nc.sync.dma_start(out=st["rt2"][D:128, :], in_=st["hr_in"][0:D, :])
            st["pt2"] = wk.tile([128, N], BF16, tag="pt2", name=f"pt2{b}")
            nc.sync.dma_start(out=st["pt2"][D:128, :], in_=st["hp_in"][0:D, :])
            st["rlt2"] = wk.tile([128, N], BF16, tag="rlt2", name=f"rlt2{b}")
            st["l_sb"] = wk.tile([128, TI, N], BF16, tag="l_sb", name=f"l_sb{b}")
            st["lt_sb"] = wk.tile([128, NI, T], BF16, tag="lt_sb", name=f"lt_sb{b}")
            st["l_dram"] = dpool.tile([T, N], BF16, tag="l_dram", name=f"l_dram{b}")
            st["lps"] = {}

        head.append(c_load)

        def c_rlt():
            # rlt2 layout: top half = RlT chunks 0,2,4,6; bottom = 1,3,5,7.
            # Each half computed by one member of a row-packed matmul pair
            # whose rhs is the even/odd-interleaved view of (replicated) Rt.
            rt_src = [st["hr_in"][0:D, :], st["rt2"][D:128, :]]
            rt_v = [
                rt_src[h].rearrange("p (c two k) -> p two c k", two=2, k=128)[:, h]
                for h in range(2)
            ]
            pss = []
            for h in range(2):
                ps = ps_mm.tile([D, 512], F32, tag="mm", name=f"rlt_ps{b}_{h}")
                nc.tensor.matmul(
                    ps[:], wl2[h * D:(h + 1) * D, :], rt_v[h],
                    tile_position=(h * D, 0),
                )
                pss.append(ps)
            for h in range(2):
                nc.vector.tensor_copy(st["rlt2"][h * D:(h + 1) * D, :], pss[h][:])

        head.append(c_rlt)

        def emit_l_pair(p):
            lp = ps_mm.tile([128, 2, N], F32, tag="mm", name=f"lps{b}_{p}")
            st["lps"][p] = lp
            for h in range(2):
                pt_src = st["hp_in"][0:D, :] if h == 0 else st["pt2"][D:128, :]
                nc.tensor.matmul(
                    lp[:, h],
                    st["rlt2"][h * D:(h + 1) * D, p * 128:(p + 1) * 128],
                    pt_src,
                    tile_position=(h * D, 0),
                )

        def emit_l_evac(p):
            nc.scalar.activation(
                st["l_sb"][:, 2 * p:2 * p + 2, :], st["lps"][p][:], AF.Tanh
            )
            l_dram_v = st["l_dram"].rearrange("(i p) n -> p i n", p=128)
            nc.sync.dma_start(
                out=l_dram_v[:, 2 * p:2 * p + 2, :],
                in_=st["l_sb"][:, 2 * p:2 * p + 2, :],
            )
            if p == 1:
                for j in range(NI):
                    nc.sync.dma_start_transpose(
                        out=st["lt_sb"][:, j, 0:512],
                        in_=st["l_dram"][0:512, j * 128:(j + 1) * 128],
                    )
            elif p == 3:
                for j in range(NI):
                    nc.sync.dma_start_transpose(
                        out=st["lt_sb"][:, j, 512:1024],
                        in_=st["l_dram"][512:1024, j * 128:(j + 1) * 128],
                    )

        def c_l_start():
            st["xps"] = ps_acc.tile([D, N], F32, tag="acc", name=f"xps{b}")
            emit_l_pair(0)
            emit_l_pair(1)
            emit_l_evac(0)

        head.append(c_l_start)

        def c_x(p):
            for i in (2 * p, 2 * p + 1):
                nc.tensor.matmul(
                    st["xps"][:], r_b[b][:, i], st["l_sb"][:, i],
                    start=(i == 0), stop=(i == TI - 1),
                )
            if p + 2 < TI // 2:
                emit_l_pair(p + 2)
            if p + 1 < TI // 2:
                emit_l_evac(p + 1)
            if p == TI // 2 - 1:
                nc.vector.tensor_copy(st["hp_in"][D:128, :], st["xps"][:])

        for p in range(TI // 2):
            head.append(lambda p=p: c_x(p))

        def t_y0():
            st["yps"] = [
                ps_acc.tile([D, 512], F32, tag="acc", name=f"yps{b}_{c}")
                for c in range(2)
            ]
            for j in range(NI):
                nc.tensor.matmul(
                    st["yps"][0][:], p_b[b][:, j], st["lt_sb"][:, j, 0:512],
                    start=(j == 0), stop=(j == NI - 1),
                )
            nc.vector.tensor_copy(st["hr_in"][D:128, 0:512], st["yps"][0][:])

        tail.append(t_y0)

        def t_hp():
            hp_bf = wk.tile([K, N], BF16, tag="hp_bf", name=f"hp_bf{b}")
            st["hp_bf"] = hp_bf
            hps = ps_acc.tile([K, N], F32, tag="acc", name=f"hps{b}")
            nc.tensor.matmul(hps[:], wt_hp[:], st["hp_in"][:])
            nc.scalar.activation(hp_bf[:], hps[:], AF.Tanh)

        tail.append(t_hp)

        def t_y1():
            for j in range(NI):
                nc.tensor.matmul(
                    st["yps"][1][:], p_b[b][:, j], st["lt_sb"][:, j, 512:1024],
                    start=(j == 0), stop=(j == NI - 1),
                )
            nc.vector.tensor_copy(st["hr_in"][D:128, 512:1024], st["yps"][1][:])

        tail.append(t_y1)

        def t_hr(c):
            if c == 0:
                st["hr_bf"] = wk.tile([K, T], BF16, tag="hr_bf", name=f"hr_bf{b}")
            hrs = ps_acc.tile([K, 512], F32, tag="acc", name=f"hrs{b}_{c}")
            nc.tensor.matmul(hrs[:], wt_hr[:], st["hr_in"][:, c * 512:(c + 1) * 512])
            nc.scalar.activation(
                st["hr_bf"][:, c * 512:(c + 1) * 512], hrs[:], AF.Tanh
            )

        tail.append(lambda: t_hr(0))
        tail.append(lambda: t_hr(1))

        def t_logits():
            lg_ps = ps_acc.tile([128, 12], F32, tag="acc", name=f"lg_ps{b}")
            for j in range(NI):
                nc.tensor.matmul(
                    lg_ps[:, j:j + 1], st["hp_bf"][:, j * 128:(j + 1) * 128],
                    whp_b[:], skip_group_check=True,
                )
            for i in range(TI):
                nc.tensor.matmul(
                    lg_ps[:, 4 + i:5 + i], st["hr_bf"][:, i * 128:(i + 1) * 128],
                    whr_b[:], skip_group_check=True,
                )
            nc.vector.tensor_copy(lgt_all[:, :, b], lg_ps[:])

        tail.append(t_logits)

        return head, tail

    prev_tail = []
    for b in range(B_LOC):
        head, tail = make_batch(b)
        hi = ti = 0
        for s in range(max(len(head), len(prev_tail))):
            if hi < len(head):
                head[hi]()
                hi += 1
            if ti < len(prev_tail):
                prev_tail[ti]()
                ti += 1
        prev_tail = tail
    for t in prev_tail:
        t()

    # ---------------- softmax phase (all batches on partitions) ----------------
    logits = inpool.tile([B_LOC, 12 * 128], F32)
    for g in range(3):
        lgt_t_ps = ps_acc.tile([B_LOC, 512], F32, tag="acc")
        for jj in range(4):
            j = g * 4 + jj
            nc.tensor.transpose(
                lgt_t_ps[:, jj * 128:(jj + 1) * 128], lgt_all[:, j, :], ident_f[:]
            )
        nc.vector.tensor_copy(logits[:, g * 512:(g + 1) * 512], lgt_t_ps[:])

    mx = inpool.tile([B_LOC, 2], F32)
    nc.vector.reduce_max(mx[:, 0:1], logits[:, 0:N], axis=mybir.AxisListType.X)
    nc.vector.reduce_max(mx[:, 1:2], logits[:, N:N + T], axis=mybir.AxisListType.X)
    nmx = inpool.tile([B_LOC, 2], F32)
    nc.vector.tensor_scalar_mul(nmx[:], mx[:], -1.0)

    probs = inpool.tile([B_LOC, 12 * 128], F32)
    sums = inpool.tile([B_LOC, 2], F32)
    nc.scalar.activation(
        probs[:, 0:N], logits[:, 0:N], AF.Exp, bias=nmx[:, 0:1], accum_out=sums[:, 0:1]
    )
    nc.scalar.activation(
        probs[:, N:N + T], logits[:, N:N + T], AF.Exp, bias=nmx[:, 1:2],
        accum_out=sums[:, 1:2],
    )
    rcp = inpool.tile([B_LOC, 2], F32)
    nc.vector.reciprocal(rcp[:], sums[:])
    pn = inpool.tile([B_LOC, 12 * 128], F32)
    nc.vector.tensor_scalar_mul(pn[:, 0:N], probs[:, 0:N], rcp[:, 0:1])
    nc.vector.tensor_scalar_mul(pn[:, N:N + T], probs[:, N:N + T], rcp[:, 1:2])

    # Transpose probs back to partition-major bf16: PrT[:, j, b]
    prt = inpool.tile([128, 12, B_LOC], BF16)
    prt_ps = ps_acc.tile([128, 12 * B_LOC], F32, tag="acc")
    for j in range(12):
        nc.tensor.transpose(
            prt_ps[:, j * B_LOC:(j + 1) * B_LOC],
            pn[:, j * 128:(j + 1) * 128],
            ident_f[0:B_LOC, 0:B_LOC],
        )
    nc.vector.tensor_copy(prt[:], prt_ps[:])

    # ---------------- pooling phase ----------------
    # co_all (64, 16): col b = P_b^T @ Ap_b, col 8+b = R_b^T @ Ar_b
    co_ps = ps_acc.tile([D, 2 * B_LOC], F32, tag="acc")
    for b in range(B_LOC):
        for j in range(NI):
            nc.tensor.matmul(
                co_ps[:, b:b + 1], p_b[b][:, j], prt[:, j, b:b + 1],
                start=(j == 0), stop=(j == NI - 1), skip_group_check=True,
            )
        for i in range(TI):
            nc.tensor.matmul(
                co_ps[:, B_LOC + b:B_LOC + b + 1], r_b[b][:, i],
                prt[:, 4 + i, b:b + 1],
                start=(i == 0), stop=(i == TI - 1), skip_group_check=True,
            )
    co_sb = inpool.tile([D, 2 * B_LOC], F32)
    nc.vector.tensor_copy(co_sb[:], co_ps[:])

    # Transpose (64, 16) -> (16, 64); row h*8+b is the h-half of out[b]
    cot_ps = ps_acc.tile([2 * B_LOC, D], F32, tag="acc")
    nc.tensor.transpose(cot_ps[:], co_sb[:], ident_f[0:D, 0:D])
    out_sb = inpool.tile([2 * B_LOC, D], F32)
    nc.vector.tensor_copy(out_sb[:], cot_ps[:])
    nc.sync.dma_start(out=out_e.ap()[:, 0:D], in_=out_sb[0:B_LOC, :])
    nc.sync.dma_start(out=out_e.ap()[:, D:2 * D], in_=out_sb[B_LOC:2 * B_LOC, :])
    ctx.close()


_NC_CACHE = None


def _get_nc():
    global _NC_CACHE
    if _NC_CACHE is None:
        _NC_CACHE = build_kernel()
    return _NC_CACHE


def _prep_host_inputs(inputs):
    import ml_dtypes

    bf = ml_dtypes.bfloat16
    rev = np.ascontiguousarray(np.asarray(inputs["review_seq"], dtype=np.float32))
    post = np.ascontiguousarray(np.asarray(inputs["post_seq"], dtype=np.float32))
    wl = np.asarray(inputs["Wl"], dtype=np.float32)
    wr = np.asarray(inputs["Wr"], dtype=np.float32)
    wp = np.asarray(inputs["Wp"], dtype=np.float32)
    whr = np.asarray(inputs["whr"], dtype=np.float32)
    whp = np.asarray(inputs["whp"], dtype=np.float32)

    rev_bf = rev.astype(bf)
    post_bf = post.astype(bf)
    B = rev.shape[0]
    # column order of the transposed layouts matches t = 8p + i / n = 4p + j:
    # rev_t[b, d, i*128 + p] = rev[b, 8p + i, d]
    rev_t = np.ascontiguousarray(
        rev_bf.reshape(B, 128, 8, 64).transpose(0, 3, 2, 1).reshape(B, 64, 1024)
    )
    post_t = np.ascontiguousarray(
        post_bf.reshape(B, 128, 4, 64).transpose(0, 3, 2, 1).reshape(B, 64, 512)
    )
    const = {
        "wl2": np.ascontiguousarray(np.concatenate([wl, wl], axis=0).astype(bf)),
        "wt_hp": np.ascontiguousarray(np.concatenate([wp.T, wr.T], axis=0).astype(bf)),
        "wt_hr": np.ascontiguousarray(np.concatenate([wr.T, wp.T], axis=0).astype(bf)),
        "whp_c": np.ascontiguousarray(whp.T.astype(bf)),
        "whr_c": np.ascontiguousarray(whr.T.astype(bf)),
        "ident": np.eye(128, dtype=np.float32),
    }
    return rev_bf, rev_t, post_bf, post_t, const


def run_on_hw(inputs: dict, trace: bool = False, **kw):
    nc = _get_nc()
    rev_bf, rev_t, post_bf, post_t, const = _prep_host_inputs(inputs)
    in_maps = []
    for c in range(NCORES):
        s = slice(c * B_LOC, (c + 1) * B_LOC)
        m = {
            "review_bf": np.ascontiguousarray(rev_bf[s]),
            "review_t": np.ascontiguousarray(rev_t[s]),
            "post_bf": np.ascontiguousarray(post_bf[s]),
            "post_t": np.ascontiguousarray(post_t[s]),
        }
        m.update(const)
        in_maps.append(m)
    res = run_bass_kernel_spmd(nc, in_maps, list(range(NCORES)), trace=trace, **kw)
    out = np.concatenate([res.results[c]["out"] for c in range(NCORES)], axis=0)
    return out, res


def kernel(**inputs) -> np.ndarray:
    out, _ = run_on_hw(inputs, trace=False)
    return out.astype(np.float32)


# revision 20
# speedup vs baseline: 1.3429x; 1.0642x over previous
"""CoAttLayer Trainium2 kernel — pure data-parallel over batch on 8 NeuronCores.

Reference computation (per batch element b, T=1024, N=512, D=64, K=80):
  L  = tanh(R @ Wl @ P^T)                    (T, N)
  Hp = tanh(Wp @ P^T + (Wr @ R^T) @ L)       (K, N)
  Hr = tanh(Wr @ R^T + (Wp @ P^T) @ L^T)     (K, T)
  Ap = softmax(whp @ Hp), Ar = softmax(whr @ Hr)
  out[b] = concat(P^T @ Ap, R^T @ Ar)        (2D,)

Reassociated into D-sized contractions:
  Hp = [Wp | Wr] @ [P^T ; X]   with X = R^T @ L    (D, N)
  Hr = [Wr | Wp] @ [R^T ; Y]   with Y = P^T @ L^T  (D, T)

Design notes (from trace analysis):
 - The PE HAM clock governor only counts real matmul activity; transpose-mode
   instructions poison it back to 1.2 GHz. So the batch loop contains ZERO PE
   transposes: all static transposed layouts (R^T, P^T, weight stacks) are
   prepared on the HOST, and the data-dependent L^T is produced by bouncing
   L through DRAM and reading it back through the DMA xbar transpose engine
   (~180 GB/s, fully off the compute engines).
 - All matmul operands are bf16 (fp32 PSUM accumulate); tanh lives on the
   Scalar engine with 1024-wide evacuations; PSUM evacuations go to DVE.
 - Softmax is batched across the 8 local batch elements on partitions.
"""

import numpy as np

import concourse.bass as bass
import concourse.bacc as bacc
import concourse.mybir as mybir
import concourse.tile as tile
from concourse.bass_utils import run_bass_kernel_spmd

F32 = mybir.dt.float32
BF16 = mybir.dt.bfloat16
AF = mybir.ActivationFunctionType

B_LOC = 8      # batch elements per core
T, N, D, K = 1024, 512, 64, 80
TI = T // 128  # 8 t-tiles
NI = N // 128  # 4 n-tiles
NCORES = 8


def build_kernel():
    nc = bacc.Bacc("TRN2", debug=False, target_bir_lowering=False)

    ins = {}
    for name, shape, dt in [
        ("review_bf", [B_LOC, T, D], BF16),
        ("review_t", [B_LOC, D, T], BF16),
        ("post_bf", [B_LOC, N, D], BF16),
        ("post_t", [B_LOC, D, N], BF16),
        ("wl2", [2 * D, D], BF16),
        ("wt_hp", [2 * D, K], BF16),
        ("wt_hr", [2 * D, K], BF16),
        ("whp_c", [K, 1], BF16),
        ("whr_c", [K, 1], BF16),
        ("ident", [128, 128], F32),
    ]:
        ins[name] = nc.declare_dram_parameter(name, shape, dt, isOutput=False)
    out_e = nc.declare_dram_parameter("out", [B_LOC, 2 * D], F32, isOutput=True)

    with tile.TileContext(nc) as tc:
        _body(nc, tc, ins, out_e)

    nc.compile()
    return nc


def _body(nc, tc, ins, out_e):
    from contextlib import ExitStack

    ctx = ExitStack()
    cpool = ctx.enter_context(tc.tile_pool(name="const", bufs=1))
    inpool = ctx.enter_context(tc.tile_pool(name="inputs", bufs=1))
    wk = ctx.enter_context(tc.tile_pool(name="work", bufs=2))
    dpool = ctx.enter_context(tc.tile_pool(name="dram", bufs=2, space="DRAM"))
    ps_mm = ctx.enter_context(tc.tile_pool(name="ps_mm", bufs=2, space="PSUM"))
    ps_acc = ctx.enter_context(tc.tile_pool(name="ps_acc", bufs=4, space="PSUM"))

    # ---------------- constants (all pre-transposed on host) ----------------
    ident_f = cpool.tile([128, 128], F32)
    nc.sync.dma_start(out=ident_f[:], in_=ins["ident"].ap())
    wl2 = cpool.tile([2 * D, D], BF16)
    nc.sync.dma_start(out=wl2[:], in_=ins["wl2"].ap())
    wt_hp = cpool.tile([2 * D, K], BF16)
    nc.sync.dma_start(out=wt_hp[:], in_=ins["wt_hp"].ap())
    wt_hr = cpool.tile([2 * D, K], BF16)
    nc.sync.dma_start(out=wt_hr[:], in_=ins["wt_hr"].ap())
    whp_b = cpool.tile([K, 1], BF16)
    nc.sync.dma_start(out=whp_b[:], in_=ins["whp_c"].ap())
    whr_b = cpool.tile([K, 1], BF16)
    nc.sync.dma_start(out=whr_b[:], in_=ins["whr_c"].ap())

    # Persistent bf16 inputs, one tile per batch (avoids false whole-tile deps)
    rbp = ctx.enter_context(tc.tile_pool(name="rbp", bufs=2 * B_LOC))
    r_b = [rbp.tile([128, TI, D], BF16, tag="r", name=f"r_b{b}") for b in range(B_LOC)]
    p_b = [rbp.tile([128, NI, D], BF16, tag="p", name=f"p_b{b}") for b in range(B_LOC)]

    # Per-batch logits, transposed layout: cols 0:4 ap n-tiles, 4:12 ar t-tiles
    lgt_all = inpool.tile([128, 12, B_LOC], F32)

    # ---------------- main compute, two global phases ----------------
    # Phase 1 (per batch): loads, RlT, L (+tanh), X, L->DRAM, LT xbar reads.
    # Phase 2 (per batch): Hp, Y, Hr, logits — consumes the LT tiles whose
    # DMA-transpose latency was hidden behind the rest of phase 1.
    # K=64 matmuls are packed two-per-issue into disjoint PE row groups
    # (K<=64 streams at half rate unpacked: 427 vs 117 ns per N=512 matmul).
    lt_pool = ctx.enter_context(tc.tile_pool(name="lt", bufs=B_LOC))
    st_all = [dict() for _ in range(B_LOC)]

    def phase1(b):
        st = st_all[b]
        nc.sync.dma_start(
            out=r_b[b][:],
            in_=ins["review_bf"].ap()[b].rearrange("(p i) d -> p i d", i=TI),
        )
        nc.sync.dma_start(
            out=p_b[b][:],
            in_=ins["post_bf"].ap()[b].rearrange("(p j) d -> p j d", j=NI),
        )
        st["hr_in"] = lt_pool.tile([128, T], BF16, tag="hr_in", name=f"hr_in{b}")
        nc.sync.dma_start(out=st["hr_in"][0:D, :], in_=ins["review_t"].ap()[b])
        st["hp_in"] = lt_pool.tile([128, N], BF16, tag="hp_in", name=f"hp_in{b}")
        nc.sync.dma_start(out=st["hp_in"][0:D, :], in_=ins["post_t"].ap()[b])
        # bottom-half replicas for the row-packed K=64 matmuls (SBUF->SBUF)
        st["rt2"] = wk.tile([128, T], BF16, tag="rt2", name=f"rt2{b}")
        nc.sync.dma_start(out=st["rt2"][D:128, :], in_=st["hr_in"][0:D, :])
        st["pt2"] = wk.tile([128, N], BF16, tag="pt2", name=f"pt2{b}")
        nc.sync.dma_start(out=st["pt2"][D:128, :], in_=st["hp_in"][0:D, :])
        st["rlt2"] = wk.tile([128, N], BF16, tag="rlt2", name=f"rlt2{b}")
        l_sb = wk.tile([128, TI, N], BF16, tag="l_sb", name=f"l_sb{b}")
        st["lt_sb"] = lt_pool.tile([128, NI, T], BF16, tag="lt", name=f"lt_sb{b}")
        l_dram = dpool.tile([T, N], BF16, tag="l_dram", name=f"l_dram{b}")
        l_dram_v = l_dram.rearrange("(i p) n -> p i n", p=128)
        lps = {}

        # rlt2 layout: top half = RlT chunks 0,2,4,6; bottom = 1,3,5,7,
        # one packed pair with even/odd interleaved views of replicated Rt.
        rt_src = [st["hr_in"][0:D, :], st["rt2"][D:128, :]]
        pss = []
        for h in range(2):
            ps = ps_mm.tile([D, 512], F32, tag="mm", name=f"rlt_ps{b}_{h}")
            rt_v = rt_src[h].rearrange("p (c two k) -> p two c k", two=2, k=128)[:, h]
            nc.tensor.matmul(
                ps[:], wl2[h * D:(h + 1) * D, :], rt_v, tile_position=(h * D, 0)
            )
            pss.append(ps)
        for h in range(2):
            nc.vector.tensor_copy(st["rlt2"][h * D:(h + 1) * D, :], pss[h][:])

        def emit_l_pair(p):
            lp = ps_mm.tile([128, 2, N], F32, tag="mm", name=f"lps{b}_{p}")
            lps[p] = lp
            for h in range(2):
                pt_src = st["hp_in"][0:D, :] if h == 0 else st["pt2"][D:128, :]
                nc.tensor.matmul(
                    lp[:, h],
                    st["rlt2"][h * D:(h + 1) * D, p * 128:(p + 1) * 128],
                    pt_src,
                    tile_position=(h * D, 0),
                )

        def emit_l_evac(p):
            nc.scalar.activation(l_sb[:, 2 * p:2 * p + 2, :], lps[p][:], AF.Tanh)
            nc.sync.dma_start(
                out=l_dram_v[:, 2 * p:2 * p + 2, :], in_=l_sb[:, 2 * p:2 * p + 2, :]
            )
            if p == 1:
                for j in range(NI):
                    nc.sync.dma_start_transpose(
                        out=st["lt_sb"][:, j, 0:512],
                        in_=l_dram[0:512, j * 128:(j + 1) * 128],
                    )
            elif p == 3:
                for j in range(NI):
                    nc.sync.dma_start_transpose(
                        out=st["lt_sb"][:, j, 512:1024],
                        in_=l_dram[512:1024, j * 128:(j + 1) * 128],
                    )

        xps = ps_acc.tile([D, N], F32, tag="acc", name=f"xps{b}")
        emit_l_pair(0)
        emit_l_pair(1)
        emit_l_evac(0)
        for p in range(TI // 2):
            for i in (2 * p, 2 * p + 1):
                nc.tensor.matmul(
                    xps[:], r_b[b][:, i], l_sb[:, i],
                    start=(i == 0), stop=(i == TI - 1),
                )
            if p + 2 < TI // 2:
                emit_l_pair(p + 2)
            if p + 1 < TI // 2:
                emit_l_evac(p + 1)
        nc.vector.tensor_copy(st["hp_in"][D:128, :], xps[:])

    def phase2(b):
        st = st_all[b]
        hp_bf = wk.tile([K, N], BF16, tag="hp_bf", name=f"hp_bf{b}")
        hps = ps_acc.tile([K, N], F32, tag="acc", name=f"hps{b}")
        nc.tensor.matmul(hps[:], wt_hp[:], st["hp_in"][:])
        nc.scalar.activation(hp_bf[:], hps[:], AF.Tanh)

        yps = [
            ps_acc.tile([D, 512], F32, tag="acc", name=f"yps{b}_{c}")
            for c in range(2)
        ]
        for c in range(2):
            for j in range(NI):
                nc.tensor.matmul(
                    yps[c][:], p_b[b][:, j],
                    st["lt_sb"][:, j, c * 512:(c + 1) * 512],
                    start=(j == 0), stop=(j == NI - 1),
                )
            nc.vector.tensor_copy(
                st["hr_in"][D:128, c * 512:(c + 1) * 512], yps[c][:]
            )

        hr_bf = wk.tile([K, T], BF16, tag="hr_bf", name=f"hr_bf{b}")
        for c in range(2):
            hrs = ps_acc.tile([K, 512], F32, tag="acc", name=f"hrs{b}_{c}")
            nc.tensor.matmul(hrs[:], wt_hr[:], st["hr_in"][:, c * 512:(c + 1) * 512])
            nc.scalar.activation(hr_bf[:, c * 512:(c + 1) * 512], hrs[:], AF.Tanh)

        lg_ps = ps_acc.tile([128, 12], F32, tag="acc", name=f"lg_ps{b}")
        for j in range(NI):
            nc.tensor.matmul(
                lg_ps[:, j:j + 1], hp_bf[:, j * 128:(j + 1) * 128], whp_b[:],
                skip_group_check=True,
            )
        for i in range(TI):
            nc.tensor.matmul(
                lg_ps[:, 4 + i:5 + i], hr_bf[:, i * 128:(i + 1) * 128], whr_b[:],
                skip_group_check=True,
            )
        nc.vector.tensor_copy(lgt_all[:, :, b], lg_ps[:])

    for b in range(B_LOC):
        phase1(b)
    for b in range(B_LOC):
        phase2(b)

    # ---------------- softmax phase (all batches on partitions) ----------------
    logits = inpool.tile([B_LOC, 12 * 128], F32)
    for g in range(3):
        lgt_t_ps = ps_acc.tile([B_LOC, 512], F32, tag="acc")
        for jj in range(4):
            j = g * 4 + jj
            nc.tensor.transpose(
                lgt_t_ps[:, jj * 128:(jj + 1) * 128], lgt_all[:, j, :], ident_f[:]
            )
        nc.vector.tensor_copy(logits[:, g * 512:(g + 1) * 512], lgt_t_ps[:])

    mx = inpool.tile([B_LOC, 2], F32)
    nc.vector.reduce_max(mx[:, 0:1], logits[:, 0:N], axis=mybir.AxisListType.X)
    nc.vector.reduce_max(mx[:, 1:2], logits[:, N:N + T], axis=mybir.AxisListType.X)
    nmx = inpool.tile([B_LOC, 2], F32)
    nc.vector.tensor_scalar_mul(nmx[:], mx[:], -1.0)

    probs = inpool.tile([B_LOC, 12 * 128], F32)
    sums = inpool.tile([B_LOC, 2], F32)
    nc.scalar.activation(
        probs[:, 0:N], logits[:, 0:N], AF.Exp, bias=nmx[:, 0:1], accum_out=sums[:, 0:1]
    )
    nc.scalar.activation(
        probs[:, N:N + T], logits[:, N:N + T], AF.Exp, bias=nmx[:, 1:2],
        accum_out=sums[:, 1:2],
    )
    rcp = inpool.tile([B_LOC, 2], F32)
    nc.vector.reciprocal(rcp[:], sums[:])
    pn = inpool.tile([B_LOC, 12 * 128], F32)
    nc.vector.tensor_scalar_mul(pn[:, 0:N], probs[:, 0:N], rcp[:, 0:1])
    nc.vector.tensor_scalar_mul(pn[:, N:N + T], probs[:, N:N + T], rcp[:, 1:2])

    # Transpose probs back to partition-major bf16: PrT[:, j, b]
    prt = inpool.tile([128, 12, B_LOC], BF16)
    prt_ps = ps_acc.tile([128, 12 * B_LOC], F32, tag="acc")
    for j in range(12):
        nc.tensor.transpose(
            prt_ps[:, j * B_LOC:(j + 1) * B_LOC],
            pn[:, j * 128:(j + 1) * 128],
            ident_f[0:B_LOC, 0:B_LOC],
        )
    nc.vector.tensor_copy(prt[:], prt_ps[:])

    # ---------------- pooling phase ----------------
    # co_all (64, 16): col b = P_b^T @ Ap_b, col 8+b = R_b^T @ Ar_b
    co_ps = ps_acc.tile([D, 2 * B_LOC], F32, tag="acc")
    for b in range(B_LOC):
        for j in range(NI):
            nc.tensor.matmul(
                co_ps[:, b:b + 1], p_b[b][:, j], prt[:, j, b:b + 1],
                start=(j == 0), stop=(j == NI - 1), skip_group_check=True,
            )
        for i in range(TI):
            nc.tensor.matmul(
                co_ps[:, B_LOC + b:B_LOC + b + 1], r_b[b][:, i],
                prt[:, 4 + i, b:b + 1],
                start=(i == 0), stop=(i == TI - 1), skip_group_check=True,
            )
    co_sb = inpool.tile([D, 2 * B_LOC], F32)
    nc.vector.tensor_copy(co_sb[:], co_ps[:])

    # Transpose (64, 16) -> (16, 64); row h*8+b is the h-half of out[b]
    cot_ps = ps_acc.tile([2 * B_LOC, D], F32, tag="acc")
    nc.tensor.transpose(cot_ps[:], co_sb[:], ident_f[0:D, 0:D])
    out_sb = inpool.tile([2 * B_LOC, D], F32)
    nc.vector.tensor_copy(out_sb[:], cot_ps[:])
    nc.sync.dma_start(out=out_e.ap()[:, 0:D], in_=out_sb[0:B_LOC, :])
    nc.sync.dma_start(out=out_e.ap()[:, D:2 * D], in_=out_sb[B_LOC:2 * B_LOC, :])
    ctx.close()


_NC_CACHE = None


def _get_nc():
    global _NC_CACHE
    if _NC_CACHE is None:
        _NC_CACHE = build_kernel()
    return _NC_CACHE


def _prep_host_inputs(inputs):
    import ml_dtypes

    bf = ml_dtypes.bfloat16
    rev = np.ascontiguousarray(np.asarray(inputs["review_seq"], dtype=np.float32))
    post = np.ascontiguousarray(np.asarray(inputs["post_seq"], dtype=np.float32))
    wl = np.asarray(inputs["Wl"], dtype=np.float32)
    wr = np.asarray(inputs["Wr"], dtype=np.float32)
    wp = np.asarray(inputs["Wp"], dtype=np.float32)
    whr = np.asarray(inputs["whr"], dtype=np.float32)
    whp = np.asarray(inputs["whp"], dtype=np.float32)

    rev_bf = rev.astype(bf)
    post_bf = post.astype(bf)
    B = rev.shape[0]
    # column order of the transposed layouts matches t = 8p + i / n = 4p + j:
    # rev_t[b, d, i*128 + p] = rev[b, 8p + i, d]
    rev_t = np.ascontiguousarray(
        rev_bf.reshape(B, 128, 8, 64).transpose(0, 3, 2, 1).reshape(B, 64, 1024)
    )
    post_t = np.ascontiguousarray(
        post_bf.reshape(B, 128, 4, 64).transpose(0, 3, 2, 1).reshape(B, 64, 512)
    )
    const = {
        "wl2": np.ascontiguousarray(np.concatenate([wl, wl], axis=0).astype(bf)),
        "wt_hp": np.ascontiguousarray(np.concatenate([wp.T, wr.T], axis=0).astype(bf)),
        "wt_hr": np.ascontiguousarray(np.concatenate([wr.T, wp.T], axis=0).astype(bf)),
        "whp_c": np.ascontiguousarray(whp.T.astype(bf)),
        "whr_c": np.ascontiguousarray(whr.T.astype(bf)),
        "ident": np.eye(128, dtype=np.float32),
    }
    return rev_bf, rev_t, post_bf, post_t, const


def run_on_hw(inputs: dict, trace: bool = False, **kw):
    nc = _get_nc()
    rev_bf, rev_t, post_bf, post_t, const = _prep_host_inputs(inputs)
    in_maps = []
    for c in range(NCORES):
        s = slice(c * B_LOC, (c + 1) * B_LOC)
        m = {
            "review_bf": np.ascontiguousarray(rev_bf[s]),
            "review_t": np.ascontiguousarray(rev_t[s]),
            "post_bf": np.ascontiguousarray(post_bf[s]),
            "post_t": np.ascontiguousarray(post_t[s]),
        }
        m.update(const)
        in_maps.append(m)
    res = run_bass_kernel_spmd(nc, in_maps, list(range(NCORES)), trace=trace, **kw)
    out = np.concatenate([res.results[c]["out"] for c in range(NCORES)], axis=0)
    return out, res


def kernel(**inputs) -> np.ndarray:
    out, _ = run_on_hw(inputs, trace=False)
    return out.astype(np.float32)


# revision 21
# speedup vs baseline: 1.3772x; 1.0255x over previous
"""CoAttLayer Trainium2 kernel — pure data-parallel over batch on 8 NeuronCores.

Reference computation (per batch element b, T=1024, N=512, D=64, K=80):
  L  = tanh(R @ Wl @ P^T)                    (T, N)
  Hp = tanh(Wp @ P^T + (Wr @ R^T) @ L)       (K, N)
  Hr = tanh(Wr @ R^T + (Wp @ P^T) @ L^T)     (K, T)
  Ap = softmax(whp @ Hp), Ar = softmax(whr @ Hr)
  out[b] = concat(P^T @ Ap, R^T @ Ar)        (2D,)

Reassociated into D-sized contractions:
  Hp = [Wp | Wr] @ [P^T ; X]   with X = R^T @ L    (D, N)
  Hr = [Wr | Wp] @ [R^T ; Y]   with Y = P^T @ L^T  (D, T)

Design notes (from trace analysis):
 - The PE HAM clock governor only counts real matmul activity; transpose-mode
   instructions poison it back to 1.2 GHz. So the batch loop contains ZERO PE
   transposes: all static transposed layouts (R^T, P^T, weight stacks) are
   prepared on the HOST, and the data-dependent L^T is produced by bouncing
   L through DRAM and reading it back through the DMA xbar transpose engine
   (~180 GB/s, fully off the compute engines).
 - All matmul operands are bf16 (fp32 PSUM accumulate); tanh lives on the
   Scalar engine with 1024-wide evacuations; PSUM evacuations go to DVE.
 - Softmax is batched across the 8 local batch elements on partitions.
"""

import numpy as np

import concourse.bass as bass
import concourse.bacc as bacc
import concourse.mybir as mybir
import concourse.tile as tile
from concourse.bass_utils import run_bass_kernel_spmd

F32 = mybir.dt.float32
BF16 = mybir.dt.bfloat16
AF = mybir.ActivationFunctionType

B_LOC = 8      # batch elements per core
T, N, D, K = 1024, 512, 64, 80
TI = T // 128  # 8 t-tiles
NI = N // 128  # 4 n-tiles
NCORES = 8


def build_kernel():
    nc = bacc.Bacc("TRN2", debug=False, target_bir_lowering=False)

    ins = {}
    for name, shape, dt in [
        ("review_bf", [B_LOC, T, D], BF16),
        ("review_t", [B_LOC, D, T], BF16),
        ("post_bf", [B_LOC, N, D], BF16),
        ("post_t", [B_LOC, D, N], BF16),
        ("wl2", [2 * D, D], BF16),
        ("wt_hp", [2 * D, K], BF16),
        ("wt_hr", [2 * D, K], BF16),
        ("whp_c", [K, 1], BF16),
        ("whr_c", [K, 1], BF16),
        ("ident", [128, 128], F32),
    ]:
        ins[name] = nc.declare_dram_parameter(name, shape, dt, isOutput=False)
    out_e = nc.declare_dram_parameter("out", [B_LOC, 2 * D], F32, isOutput=True)

    with tile.TileContext(nc) as tc:
        _body(nc, tc, ins, out_e)

    nc.compile()
    return nc


def _body(nc, tc, ins, out_e):
    from contextlib import ExitStack

    ctx = ExitStack()
    cpool = ctx.enter_context(tc.tile_pool(name="const", bufs=1))
    inpool = ctx.enter_context(tc.tile_pool(name="inputs", bufs=1))
    wk = ctx.enter_context(tc.tile_pool(name="work", bufs=2))
    dpool = ctx.enter_context(tc.tile_pool(name="dram", bufs=2, space="DRAM"))
    ps_mm = ctx.enter_context(tc.tile_pool(name="ps_mm", bufs=2, space="PSUM"))
    ps_acc = ctx.enter_context(tc.tile_pool(name="ps_acc", bufs=4, space="PSUM"))

    # ---------------- constants (all pre-transposed on host) ----------------
    ident_f = cpool.tile([128, 128], F32)
    nc.sync.dma_start(out=ident_f[:], in_=ins["ident"].ap())
    wl2 = cpool.tile([2 * D, D], BF16)
    nc.sync.dma_start(out=wl2[:], in_=ins["wl2"].ap())
    wt_hp = cpool.tile([2 * D, K], BF16)
    nc.sync.dma_start(out=wt_hp[:], in_=ins["wt_hp"].ap())
    wt_hr = cpool.tile([2 * D, K], BF16)
    nc.sync.dma_start(out=wt_hr[:], in_=ins["wt_hr"].ap())
    whp_b = cpool.tile([K, 1], BF16)
    nc.sync.dma_start(out=whp_b[:], in_=ins["whp_c"].ap())
    whr_b = cpool.tile([K, 1], BF16)
    nc.sync.dma_start(out=whr_b[:], in_=ins["whr_c"].ap())

    # Persistent bf16 inputs, one tile per batch (avoids false whole-tile deps)
    rbp = ctx.enter_context(tc.tile_pool(name="rbp", bufs=2 * B_LOC))
    r_b = [rbp.tile([128, TI, D], BF16, tag="r", name=f"r_b{b}") for b in range(B_LOC)]
    p_b = [rbp.tile([128, NI, D], BF16, tag="p", name=f"p_b{b}") for b in range(B_LOC)]

    # Per-batch logits, transposed layout: cols 0:4 ap n-tiles, 4:12 ar t-tiles
    lgt_all = inpool.tile([128, 12, B_LOC], F32)

    # ---------------- main compute, two global phases ----------------
    # Phase 1 (per batch): loads, RlT, L (+tanh), X, L->DRAM, LT xbar reads.
    # Phase 2 (per batch): Hp, Y, Hr, logits — consumes the LT tiles whose
    # DMA-transpose latency was hidden behind the rest of phase 1.
    # K=64 matmuls are packed two-per-issue into disjoint PE row groups
    # (K<=64 streams at half rate unpacked: 427 vs 117 ns per N=512 matmul).
    lt_pool = ctx.enter_context(tc.tile_pool(name="lt", bufs=B_LOC))
    st_all = [dict() for _ in range(B_LOC)]

    # Hoist every DRAM input load to the top so the in-order DMA sequencer
    # never blocks a later batch's load behind a data-dependent wait.
    for b in range(B_LOC):
        st = st_all[b]
        nc.sync.dma_start(
            out=r_b[b][:],
            in_=ins["review_bf"].ap()[b].rearrange("(p i) d -> p i d", i=TI),
        )
        nc.sync.dma_start(
            out=p_b[b][:],
            in_=ins["post_bf"].ap()[b].rearrange("(p j) d -> p j d", j=NI),
        )
        # [Rt ; Rt] — bottom replica feeds the row-packed K=64 matmuls and is
        # overwritten by Y in phase 2 (likewise [Pt ; Pt] / X).
        st["hr_in"] = lt_pool.tile([128, T], BF16, tag="hr_in", name=f"hr_in{b}")
        nc.sync.dma_start(out=st["hr_in"][0:D, :], in_=ins["review_t"].ap()[b])
        nc.sync.dma_start(out=st["hr_in"][D:128, :], in_=ins["review_t"].ap()[b])
        st["hp_in"] = lt_pool.tile([128, N], BF16, tag="hp_in", name=f"hp_in{b}")
        nc.sync.dma_start(out=st["hp_in"][0:D, :], in_=ins["post_t"].ap()[b])
        nc.sync.dma_start(out=st["hp_in"][D:128, :], in_=ins["post_t"].ap()[b])

    pending_lt = []

    def phase1(b):
        st = st_all[b]
        # flush previous batch's transposed-LT reads (their producer DMAs have
        # long completed, so the sequencer wait is satisfied immediately)
        for fn in pending_lt:
            fn()
        pending_lt.clear()

        st["rlt2"] = wk.tile([128, N], BF16, tag="rlt2", name=f"rlt2{b}")
        l_sb = wk.tile([128, TI, N], BF16, tag="l_sb", name=f"l_sb{b}")
        st["lt_sb"] = lt_pool.tile([128, NI, T], BF16, tag="lt", name=f"lt_sb{b}")
        l_dram = dpool.tile([T, N], BF16, tag="l_dram", name=f"l_dram{b}")
        l_dram_v = l_dram.rearrange("(i p) n -> p i n", p=128)
        lps = {}

        # rlt2 layout: top half = RlT chunks 0,2,4,6; bottom = 1,3,5,7,
        # one packed pair with even/odd interleaved views of replicated Rt.
        pss = []
        for h in range(2):
            ps = ps_mm.tile([D, 512], F32, tag="mm", name=f"rlt_ps{b}_{h}")
            rt_v = st["hr_in"][h * D:(h + 1) * D, :].rearrange(
                "p (c two k) -> p two c k", two=2, k=128
            )[:, h]
            nc.tensor.matmul(
                ps[:], wl2[h * D:(h + 1) * D, :], rt_v, tile_position=(h * D, 0)
            )
            pss.append(ps)
        for h in range(2):
            nc.vector.tensor_copy(st["rlt2"][h * D:(h + 1) * D, :], pss[h][:])

        def emit_l_pair(p):
            lp = ps_mm.tile([128, 2, N], F32, tag="mm", name=f"lps{b}_{p}")
            lps[p] = lp
            for h in range(2):
                nc.tensor.matmul(
                    lp[:, h],
                    st["rlt2"][h * D:(h + 1) * D, p * 128:(p + 1) * 128],
                    st["hp_in"][h * D:(h + 1) * D, :],
                    tile_position=(h * D, 0),
                )

        def emit_l_evac(p):
            nc.scalar.activation(l_sb[:, 2 * p:2 * p + 2, :], lps[p][:], AF.Tanh)
            nc.sync.dma_start(
                out=l_dram_v[:, 2 * p:2 * p + 2, :], in_=l_sb[:, 2 * p:2 * p + 2, :]
            )
            if p == 1:
                def rd_lo():
                    for j in range(NI):
                        nc.sync.dma_start_transpose(
                            out=st["lt_sb"][:, j, 0:512],
                            in_=l_dram[0:512, j * 128:(j + 1) * 128],
                        )
                pending_lt.append(rd_lo)
            elif p == 3:
                def rd_hi():
                    for j in range(NI):
                        nc.sync.dma_start_transpose(
                            out=st["lt_sb"][:, j, 512:1024],
                            in_=l_dram[512:1024, j * 128:(j + 1) * 128],
                        )
                pending_lt.append(rd_hi)

        xps = ps_acc.tile([D, N], F32, tag="acc", name=f"xps{b}")
        emit_l_pair(0)
        emit_l_pair(1)
        emit_l_evac(0)
        for p in range(TI // 2):
            for i in (2 * p, 2 * p + 1):
                nc.tensor.matmul(
                    xps[:], r_b[b][:, i], l_sb[:, i],
                    start=(i == 0), stop=(i == TI - 1),
                )
            if p + 2 < TI // 2:
                emit_l_pair(p + 2)
            if p + 1 < TI // 2:
                emit_l_evac(p + 1)
        nc.vector.tensor_copy(st["hp_in"][D:128, :], xps[:])

    def phase2(b):
        st = st_all[b]
        hp_bf = wk.tile([K, N], BF16, tag="hp_bf", name=f"hp_bf{b}")
        hps = ps_acc.tile([K, N], F32, tag="acc", name=f"hps{b}")
        nc.tensor.matmul(hps[:], wt_hp[:], st["hp_in"][:])
        nc.scalar.activation(hp_bf[:], hps[:], AF.Tanh)

        yps = [
            ps_acc.tile([D, 512], F32, tag="acc", name=f"yps{b}_{c}")
            for c in range(2)
        ]
        for c in range(2):
            for j in range(NI):
                nc.tensor.matmul(
                    yps[c][:], p_b[b][:, j],
                    st["lt_sb"][:, j, c * 512:(c + 1) * 512],
                    start=(j == 0), stop=(j == NI - 1),
                )
            nc.vector.tensor_copy(
                st["hr_in"][D:128, c * 512:(c + 1) * 512], yps[c][:]
            )

        hr_bf = wk.tile([K, T], BF16, tag="hr_bf", name=f"hr_bf{b}")
        for c in range(2):
            hrs = ps_acc.tile([K, 512], F32, tag="acc", name=f"hrs{b}_{c}")
            nc.tensor.matmul(hrs[:], wt_hr[:], st["hr_in"][:, c * 512:(c + 1) * 512])
            nc.scalar.activation(hr_bf[:, c * 512:(c + 1) * 512], hrs[:], AF.Tanh)

        lg_ps = ps_acc.tile([128, 12], F32, tag="acc", name=f"lg_ps{b}")
        for j in range(NI):
            nc.tensor.matmul(
                lg_ps[:, j:j + 1], hp_bf[:, j * 128:(j + 1) * 128], whp_b[:],
                skip_group_check=True,
            )
        for i in range(TI):
            nc.tensor.matmul(
                lg_ps[:, 4 + i:5 + i], hr_bf[:, i * 128:(i + 1) * 128], whr_b[:],
                skip_group_check=True,
            )
        nc.vector.tensor_copy(lgt_all[:, :, b], lg_ps[:])

    for b in range(B_LOC):
        phase1(b)
    for fn in pending_lt:
        fn()
    pending_lt.clear()
    for b in range(B_LOC):
        phase2(b)

    # ---------------- softmax phase (all batches on partitions) ----------------
    logits = inpool.tile([B_LOC, 12 * 128], F32)
    for g in range(3):
        lgt_t_ps = ps_acc.tile([B_LOC, 512], F32, tag="acc")
        for jj in range(4):
            j = g * 4 + jj
            nc.tensor.transpose(
                lgt_t_ps[:, jj * 128:(jj + 1) * 128], lgt_all[:, j, :], ident_f[:]
            )
        nc.vector.tensor_copy(logits[:, g * 512:(g + 1) * 512], lgt_t_ps[:])

    mx = inpool.tile([B_LOC, 2], F32)
    nc.vector.reduce_max(mx[:, 0:1], logits[:, 0:N], axis=mybir.AxisListType.X)
    nc.vector.reduce_max(mx[:, 1:2], logits[:, N:N + T], axis=mybir.AxisListType.X)
    nmx = inpool.tile([B_LOC, 2], F32)
    nc.vector.tensor_scalar_mul(nmx[:], mx[:], -1.0)

    probs = inpool.tile([B_LOC, 12 * 128], F32)
    sums = inpool.tile([B_LOC, 2], F32)
    nc.scalar.activation(
        probs[:, 0:N], logits[:, 0:N], AF.Exp, bias=nmx[:, 0:1], accum_out=sums[:, 0:1]
    )
    nc.scalar.activation(
        probs[:, N:N + T], logits[:, N:N + T], AF.Exp, bias=nmx[:, 1:2],
        accum_out=sums[:, 1:2],
    )
    rcp = inpool.tile([B_LOC, 2], F32)
    nc.vector.reciprocal(rcp[:], sums[:])
    pn = inpool.tile([B_LOC, 12 * 128], F32)
    nc.vector.tensor_scalar_mul(pn[:, 0:N], probs[:, 0:N], rcp[:, 0:1])
    nc.vector.tensor_scalar_mul(pn[:, N:N + T], probs[:, N:N + T], rcp[:, 1:2])

    # Transpose probs back to partition-major bf16: PrT[:, j, b]
    prt = inpool.tile([128, 12, B_LOC], BF16)
    prt_ps = ps_acc.tile([128, 12 * B_LOC], F32, tag="acc")
    for j in range(12):
        nc.tensor.transpose(
            prt_ps[:, j * B_LOC:(j + 1) * B_LOC],
            pn[:, j * 128:(j + 1) * 128],
            ident_f[0:B_LOC, 0:B_LOC],
        )
    nc.vector.tensor_copy(prt[:], prt_ps[:])

    # ---------------- pooling phase ----------------
    # co_all (64, 16): col b = P_b^T @ Ap_b, col 8+b = R_b^T @ Ar_b
    co_ps = ps_acc.tile([D, 2 * B_LOC], F32, tag="acc")
    for b in range(B_LOC):
        for j in range(NI):
            nc.tensor.matmul(
                co_ps[:, b:b + 1], p_b[b][:, j], prt[:, j, b:b + 1],
                start=(j == 0), stop=(j == NI - 1), skip_group_check=True,
            )
        for i in range(TI):
            nc.tensor.matmul(
                co_ps[:, B_LOC + b:B_LOC + b + 1], r_b[b][:, i],
                prt[:, 4 + i, b:b + 1],
                start=(i == 0), stop=(i == TI - 1), skip_group_check=True,
            )
    co_sb = inpool.tile([D, 2 * B_LOC], F32)
    nc.vector.tensor_copy(co_sb[:], co_ps[:])

    # Transpose (64, 16) -> (16, 64); row h*8+b is the h-half of out[b]
    cot_ps = ps_acc.tile([2 * B_LOC, D], F32, tag="acc")
    nc.tensor.transpose(cot_ps[:], co_sb[:], ident_f[0:D, 0:D])
    out_sb = inpool.tile([2 * B_LOC, D], F32)
    nc.vector.tensor_copy(out_sb[:], cot_ps[:])
    nc.sync.dma_start(out=out_e.ap()[:, 0:D], in_=out_sb[0:B_LOC, :])
    nc.sync.dma_start(out=out_e.ap()[:, D:2 * D], in_=out_sb[B_LOC:2 * B_LOC, :])
    ctx.close()


_NC_CACHE = None


def _get_nc():
    global _NC_CACHE
    if _NC_CACHE is None:
        _NC_CACHE = build_kernel()
    return _NC_CACHE


def _prep_host_inputs(inputs):
    import ml_dtypes

    bf = ml_dtypes.bfloat16
    rev = np.ascontiguousarray(np.asarray(inputs["review_seq"], dtype=np.float32))
    post = np.ascontiguousarray(np.asarray(inputs["post_seq"], dtype=np.float32))
    wl = np.asarray(inputs["Wl"], dtype=np.float32)
    wr = np.asarray(inputs["Wr"], dtype=np.float32)
    wp = np.asarray(inputs["Wp"], dtype=np.float32)
    whr = np.asarray(inputs["whr"], dtype=np.float32)
    whp = np.asarray(inputs["whp"], dtype=np.float32)

    rev_bf = rev.astype(bf)
    post_bf = post.astype(bf)
    B = rev.shape[0]
    # column order of the transposed layouts matches t = 8p + i / n = 4p + j:
    # rev_t[b, d, i*128 + p] = rev[b, 8p + i, d]
    rev_t = np.ascontiguousarray(
        rev_bf.reshape(B, 128, 8, 64).transpose(0, 3, 2, 1).reshape(B, 64, 1024)
    )
    post_t = np.ascontiguousarray(
        post_bf.reshape(B, 128, 4, 64).transpose(0, 3, 2, 1).reshape(B, 64, 512)
    )
    const = {
        "wl2": np.ascontiguousarray(np.concatenate([wl, wl], axis=0).astype(bf)),
        "wt_hp": np.ascontiguousarray(np.concatenate([wp.T, wr.T], axis=0).astype(bf)),
        "wt_hr": np.ascontiguousarray(np.concatenate([wr.T, wp.T], axis=0).astype(bf)),
        "whp_c": np.ascontiguousarray(whp.T.astype(bf)),
        "whr_c": np.ascontiguousarray(whr.T.astype(bf)),
        "ident": np.eye(128, dtype=np.float32),
    }
    return rev_bf, rev_t, post_bf, post_t, const


def run_on_hw(inputs: dict, trace: bool = False, **kw):
    nc = _get_nc()
    rev_bf, rev_t, post_bf, post_t, const = _prep_host_inputs(inputs)
    in_maps = []
    for c in range(NCORES):
        s = slice(c * B_LOC, (c + 1) * B_LOC)
        m = {
            "review_bf": np.ascontiguousarray(rev_bf[s]),
            "review_t": np.ascontiguousarray(rev_t[s]),
            "post_bf": np.ascontiguousarray(post_bf[s]),
            "post_t": np.ascontiguousarray(post_t[s]),
        }
        m.update(const)
        in_maps.append(m)
    res = run_bass_kernel_spmd(nc, in_maps, list(range(NCORES)), trace=trace, **kw)
    out = np.concatenate([res.results[c]["out"] for c in range(NCORES)], axis=0)
    return out, res


def kernel(**inputs) -> np.ndarray:
    out, _ = run_on_hw(inputs, trace=False)
    return out.astype(np.float32)


# revision 23
# speedup vs baseline: 1.7652x; 1.2817x over previous
"""CoAttLayer Trainium2 kernel — pure data-parallel over batch on 8 NeuronCores.

Reference computation (per batch element b, T=1024, N=512, D=64, K=80):
  L  = tanh(R @ Wl @ P^T)                    (T, N)
  Hp = tanh(Wp @ P^T + (Wr @ R^T) @ L)       (K, N)
  Hr = tanh(Wr @ R^T + (Wp @ P^T) @ L^T)     (K, T)
  Ap = softmax(whp @ Hp), Ar = softmax(whr @ Hr)
  out[b] = concat(P^T @ Ap, R^T @ Ar)        (2D,)

Reassociated into D-sized contractions:
  Hp = [Wp | Wr] @ [P^T ; X]   with X = R^T @ L    (D, N)
  Hr = [Wr | Wp] @ [R^T ; Y]   with Y = P^T @ L^T  (D, T)

Design notes (from trace analysis):
 - The PE HAM clock governor only counts real matmul activity; transpose-mode
   instructions poison it back to 1.2 GHz. So the batch loop contains ZERO PE
   transposes: all static transposed layouts (R^T, P^T, weight stacks) are
   prepared on the HOST, and the data-dependent L^T is produced by bouncing
   L through DRAM and reading it back through the DMA xbar transpose engine
   (~180 GB/s, fully off the compute engines).
 - All matmul operands are bf16 (fp32 PSUM accumulate); tanh lives on the
   Scalar engine with 1024-wide evacuations; PSUM evacuations go to DVE.
 - Softmax is batched across the 8 local batch elements on partitions.
"""

import numpy as np

import concourse.bass as bass
import concourse.bacc as bacc
import concourse.mybir as mybir
import concourse.tile as tile
from concourse.bass_utils import run_bass_kernel_spmd

F32 = mybir.dt.float32
BF16 = mybir.dt.bfloat16
AF = mybir.ActivationFunctionType

B_LOC = 8      # batch elements per core
T, N, D, K = 1024, 512, 64, 80
TI = T // 128  # 8 t-tiles
NI = N // 128  # 4 n-tiles
NCORES = 8


def build_kernel():
    nc = bacc.Bacc("TRN2", debug=False, target_bir_lowering=False)

    ins = {}
    for name, shape, dt in [
        ("review_bf", [B_LOC, T, D], BF16),
        ("review_t", [B_LOC, D, T], BF16),
        ("post_bf", [B_LOC, N, D], BF16),
        ("post_t", [B_LOC, D, N], BF16),
        ("wl2", [2 * D, D], BF16),
        ("wt_hp", [2 * D, K], BF16),
        ("wt_hr", [2 * D, K], BF16),
        ("whp_c", [K, 1], BF16),
        ("whr_c", [K, 1], BF16),
        ("ident", [128, 128], F32),
    ]:
        ins[name] = nc.declare_dram_parameter(name, shape, dt, isOutput=False)
    out_e = nc.declare_dram_parameter("out", [B_LOC, 2 * D], F32, isOutput=True)

    with tile.TileContext(nc) as tc:
        _body(nc, tc, ins, out_e)

    nc.compile()
    return nc


def _body(nc, tc, ins, out_e):
    from contextlib import ExitStack

    ctx = ExitStack()
    cpool = ctx.enter_context(tc.tile_pool(name="const", bufs=1))
    inpool = ctx.enter_context(tc.tile_pool(name="inputs", bufs=1))
    wk = ctx.enter_context(tc.tile_pool(name="work", bufs=2))
    ps_mm = ctx.enter_context(tc.tile_pool(name="ps_mm", bufs=2, space="PSUM"))
    ps_acc = ctx.enter_context(tc.tile_pool(name="ps_acc", bufs=2, space="PSUM"))

    # ---------------- constants (all pre-transposed on host) ----------------
    ident_f = cpool.tile([128, 128], F32)
    nc.sync.dma_start(out=ident_f[:], in_=ins["ident"].ap())
    ident_b = cpool.tile([128, 128], BF16)
    nc.vector.tensor_copy(ident_b[:], ident_f[:])
    wl2 = cpool.tile([2 * D, D], BF16)
    nc.sync.dma_start(out=wl2[:], in_=ins["wl2"].ap())
    wt_hp = cpool.tile([2 * D, K], BF16)
    nc.sync.dma_start(out=wt_hp[:], in_=ins["wt_hp"].ap())
    wt_hr = cpool.tile([2 * D, K], BF16)
    nc.sync.dma_start(out=wt_hr[:], in_=ins["wt_hr"].ap())
    whp_b = cpool.tile([K, 1], BF16)
    nc.sync.dma_start(out=whp_b[:], in_=ins["whp_c"].ap())
    whr_b = cpool.tile([K, 1], BF16)
    nc.sync.dma_start(out=whr_b[:], in_=ins["whr_c"].ap())

    # Persistent bf16 inputs (written once by merged DMAs, then read-only)
    r_ball = inpool.tile([128, B_LOC, TI, D], BF16)
    p_ball = inpool.tile([128, B_LOC, NI, D], BF16)

    # Per-batch logits, transposed layout: cols 0:4 ap n-tiles, 4:12 ar t-tiles
    lgt_all = inpool.tile([128, 12, B_LOC], F32)

    # ---------------- main compute, two global phases ----------------
    # Phase 1 (per batch): loads, RlT, L (+tanh), X, L->DRAM, LT xbar reads.
    # Phase 2 (per batch): Hp, Y, Hr, logits — consumes the LT tiles whose
    # DMA-transpose latency was hidden behind the rest of phase 1.
    # K=64 matmuls are packed two-per-issue into disjoint PE row groups
    # (K<=64 streams at half rate unpacked: 427 vs 117 ns per N=512 matmul).
    lt_pool = ctx.enter_context(tc.tile_pool(name="lt", bufs=B_LOC))
    ps_tp = ctx.enter_context(tc.tile_pool(name="ps_tp", bufs=2, space="PSUM"))
    st_all = [dict() for _ in range(B_LOC)]

    # Merged input loads: one HWDGE trigger per tensor (the per-trigger cost
    # on the in-order Sync sequencer is ~0.7us — keep the count tiny).
    nc.sync.dma_start(
        out=r_ball[:],
        in_=ins["review_bf"].ap().rearrange("b (p i) d -> p b i d", i=TI),
    )
    nc.sync.dma_start(
        out=p_ball[:],
        in_=ins["post_bf"].ap().rearrange("b (p j) d -> p b j d", j=NI),
    )
    hr_all = inpool.tile([128, B_LOC, T], BF16)
    hp_all = inpool.tile([128, B_LOC, N], BF16)
    for h in range(2):
        nc.sync.dma_start(
            out=hr_all[h * D:(h + 1) * D, :, :],
            in_=ins["review_t"].ap().rearrange("b d t -> d b t"),
        )
        nc.sync.dma_start(
            out=hp_all[h * D:(h + 1) * D, :, :],
            in_=ins["post_t"].ap().rearrange("b d t -> d b t"),
        )

    def phase1(b):
        st = st_all[b]
        st["hr_in"] = hr_all[:, b, :]
        st["hp_in"] = hp_all[:, b, :]
        st["rlt2"] = wk.tile([128, N], BF16, tag="rlt2", name=f"rlt2{b}")
        l_sb = wk.tile([128, TI, N], BF16, tag="l_sb", name=f"l_sb{b}")
        st["lt_sb"] = lt_pool.tile([128, NI, T], BF16, tag="lt", name=f"lt_sb{b}")
        lps = {}

        # rlt2 layout: top half = RlT chunks 0,2,4,6; bottom = 1,3,5,7,
        # one packed pair with even/odd interleaved views of replicated Rt.
        pss = []
        for h in range(2):
            ps = ps_mm.tile([D, 512], F32, tag="mm", name=f"rlt_ps{b}_{h}")
            rt_v = st["hr_in"][h * D:(h + 1) * D, :].rearrange(
                "p (c two k) -> p two c k", two=2, k=128
            )[:, h]
            nc.tensor.matmul(
                ps[:], wl2[h * D:(h + 1) * D, :], rt_v, tile_position=(h * D, 0)
            )
            pss.append(ps)
        for h in range(2):
            nc.scalar.copy(st["rlt2"][h * D:(h + 1) * D, :], pss[h][:])

        def emit_l_pair(p):
            lp = ps_mm.tile([128, 2, N], F32, tag="mm", name=f"lps{b}_{p}")
            lps[p] = lp
            for h in range(2):
                nc.tensor.matmul(
                    lp[:, h],
                    st["rlt2"][h * D:(h + 1) * D, p * 128:(p + 1) * 128],
                    st["hp_in"][h * D:(h + 1) * D, :],
                    tile_position=(h * D, 0),
                )

        def emit_l_evac(p):
            nc.scalar.activation(l_sb[:, 2 * p:2 * p + 2, :], lps[p][:], AF.Tanh)

        def emit_lt_pair(p):
            # PE block-transposes of the tanh'd pair into one 1-bank PSUM
            # tile, then a single wide DVE evacuation into lt_sb.
            tp = ps_tp.tile([128, NI, 2, 128], BF16, tag="tp", name=f"tp{b}_{p}")
            for j in range(NI):
                for h in range(2):
                    nc.tensor.transpose(
                        tp[:, j, h],
                        l_sb[:, 2 * p + h, j * 128:(j + 1) * 128],
                        ident_b[:],
                    )
            nc.vector.tensor_copy(
                st["lt_sb"][:, :, 2 * p * 128:(2 * p + 2) * 128]
                .rearrange("q j (two k) -> q j two k", k=128),
                tp[:],
            )

        xps = ps_acc.tile([D, N], F32, tag="acc", name=f"xps{b}")
        emit_l_pair(0)
        emit_l_pair(1)
        emit_l_evac(0)
        for p in range(TI // 2):
            for i in (2 * p, 2 * p + 1):
                nc.tensor.matmul(
                    xps[:], r_ball[:, b, i], l_sb[:, i],
                    start=(i == 0), stop=(i == TI - 1),
                )
            if p + 2 < TI // 2:
                emit_l_pair(p + 2)
            if p + 1 < TI // 2:
                emit_l_evac(p + 1)
            emit_lt_pair(p)
        nc.vector.tensor_copy(st["hp_in"][D:128, :], xps[:])

    def phase2(b):
        st = st_all[b]
        hp_bf = wk.tile([K, N], BF16, tag="hp_bf", name=f"hp_bf{b}")
        hps = ps_acc.tile([K, N], F32, tag="acc", name=f"hps{b}")
        nc.tensor.matmul(hps[:], wt_hp[:], st["hp_in"][:])
        nc.scalar.activation(hp_bf[:], hps[:], AF.Tanh)

        yps = [
            ps_acc.tile([D, 512], F32, tag="acc", name=f"yps{b}_{c}")
            for c in range(2)
        ]
        for c in range(2):
            for j in range(NI):
                nc.tensor.matmul(
                    yps[c][:], p_ball[:, b, j],
                    st["lt_sb"][:, j, c * 512:(c + 1) * 512],
                    start=(j == 0), stop=(j == NI - 1),
                )
            nc.vector.tensor_copy(
                st["hr_in"][D:128, c * 512:(c + 1) * 512], yps[c][:]
            )

        hr_bf = wk.tile([K, T], BF16, tag="hr_bf", name=f"hr_bf{b}")
        for c in range(2):
            hrs = ps_acc.tile([K, 512], F32, tag="acc", name=f"hrs{b}_{c}")
            nc.tensor.matmul(hrs[:], wt_hr[:], st["hr_in"][:, c * 512:(c + 1) * 512])
            nc.scalar.activation(hr_bf[:, c * 512:(c + 1) * 512], hrs[:], AF.Tanh)

        lg_ps = ps_acc.tile([128, 12], F32, tag="acc", name=f"lg_ps{b}")
        for j in range(NI):
            nc.tensor.matmul(
                lg_ps[:, j:j + 1], hp_bf[:, j * 128:(j + 1) * 128], whp_b[:],
                skip_group_check=True,
            )
        for i in range(TI):
            nc.tensor.matmul(
                lg_ps[:, 4 + i:5 + i], hr_bf[:, i * 128:(i + 1) * 128], whr_b[:],
                skip_group_check=True,
            )
        nc.vector.tensor_copy(lgt_all[:, :, b], lg_ps[:])

    for b in range(B_LOC):
        phase1(b)
    for b in range(B_LOC):
        phase2(b)

    # ---------------- softmax phase (all batches on partitions) ----------------
    logits = inpool.tile([B_LOC, 12 * 128], F32)
    for g in range(3):
        lgt_t_ps = ps_acc.tile([B_LOC, 512], F32, tag="acc")
        for jj in range(4):
            j = g * 4 + jj
            nc.tensor.transpose(
                lgt_t_ps[:, jj * 128:(jj + 1) * 128], lgt_all[:, j, :], ident_f[:]
            )
        nc.vector.tensor_copy(logits[:, g * 512:(g + 1) * 512], lgt_t_ps[:])

    mx = inpool.tile([B_LOC, 2], F32)
    nc.vector.reduce_max(mx[:, 0:1], logits[:, 0:N], axis=mybir.AxisListType.X)
    nc.vector.reduce_max(mx[:, 1:2], logits[:, N:N + T], axis=mybir.AxisListType.X)
    nmx = inpool.tile([B_LOC, 2], F32)
    nc.vector.tensor_scalar_mul(nmx[:], mx[:], -1.0)

    probs = inpool.tile([B_LOC, 12 * 128], F32)
    sums = inpool.tile([B_LOC, 2], F32)
    nc.scalar.activation(
        probs[:, 0:N], logits[:, 0:N], AF.Exp, bias=nmx[:, 0:1], accum_out=sums[:, 0:1]
    )
    nc.scalar.activation(
        probs[:, N:N + T], logits[:, N:N + T], AF.Exp, bias=nmx[:, 1:2],
        accum_out=sums[:, 1:2],
    )
    rcp = inpool.tile([B_LOC, 2], F32)
    nc.vector.reciprocal(rcp[:], sums[:])
    pn = inpool.tile([B_LOC, 12 * 128], F32)
    nc.vector.tensor_scalar_mul(pn[:, 0:N], probs[:, 0:N], rcp[:, 0:1])
    nc.vector.tensor_scalar_mul(pn[:, N:N + T], probs[:, N:N + T], rcp[:, 1:2])

    # Transpose probs back to partition-major bf16: PrT[:, j, b]
    prt = inpool.tile([128, 12, B_LOC], BF16)
    prt_ps = ps_acc.tile([128, 12 * B_LOC], F32, tag="acc")
    for j in range(12):
        nc.tensor.transpose(
            prt_ps[:, j * B_LOC:(j + 1) * B_LOC],
            pn[:, j * 128:(j + 1) * 128],
            ident_f[0:B_LOC, 0:B_LOC],
        )
    nc.vector.tensor_copy(prt[:], prt_ps[:])

    # ---------------- pooling phase ----------------
    # co_all (64, 16): col b = P_b^T @ Ap_b, col 8+b = R_b^T @ Ar_b
    co_ps = ps_acc.tile([D, 2 * B_LOC], F32, tag="acc")
    for b in range(B_LOC):
        for j in range(NI):
            nc.tensor.matmul(
                co_ps[:, b:b + 1], p_ball[:, b, j], prt[:, j, b:b + 1],
                start=(j == 0), stop=(j == NI - 1), skip_group_check=True,
            )
        for i in range(TI):
            nc.tensor.matmul(
                co_ps[:, B_LOC + b:B_LOC + b + 1], r_ball[:, b, i],
                prt[:, 4 + i, b:b + 1],
                start=(i == 0), stop=(i == TI - 1), skip_group_check=True,
            )
    co_sb = inpool.tile([D, 2 * B_LOC], F32)
    nc.vector.tensor_copy(co_sb[:], co_ps[:])

    # Transpose (64, 16) -> (16, 64); row h*8+b is the h-half of out[b]
    cot_ps = ps_acc.tile([2 * B_LOC, D], F32, tag="acc")
    nc.tensor.transpose(cot_ps[:], co_sb[:], ident_f[0:D, 0:D])
    out_sb = inpool.tile([2 * B_LOC, D], F32)
    nc.vector.tensor_copy(out_sb[:], cot_ps[:])
    nc.sync.dma_start(out=out_e.ap()[:, 0:D], in_=out_sb[0:B_LOC, :])
    nc.sync.dma_start(out=out_e.ap()[:, D:2 * D], in_=out_sb[B_LOC:2 * B_LOC, :])
    ctx.close()


_NC_CACHE = None


def _get_nc():
    global _NC_CACHE
    if _NC_CACHE is None:
        _NC_CACHE = build_kernel()
    return _NC_CACHE


def _prep_host_inputs(inputs):
    import ml_dtypes

    bf = ml_dtypes.bfloat16
    rev = np.ascontiguousarray(np.asarray(inputs["review_seq"], dtype=np.float32))
    post = np.ascontiguousarray(np.asarray(inputs["post_seq"], dtype=np.float32))
    wl = np.asarray(inputs["Wl"], dtype=np.float32)
    wr = np.asarray(inputs["Wr"], dtype=np.float32)
    wp = np.asarray(inputs["Wp"], dtype=np.float32)
    whr = np.asarray(inputs["whr"], dtype=np.float32)
    whp = np.asarray(inputs["whp"], dtype=np.float32)

    rev_bf = rev.astype(bf)
    post_bf = post.astype(bf)
    B = rev.shape[0]
    # column order of the transposed layouts matches t = 8p + i / n = 4p + j:
    # rev_t[b, d, i*128 + p] = rev[b, 8p + i, d]
    rev_t = np.ascontiguousarray(
        rev_bf.reshape(B, 128, 8, 64).transpose(0, 3, 2, 1).reshape(B, 64, 1024)
    )
    post_t = np.ascontiguousarray(
        post_bf.reshape(B, 128, 4, 64).transpose(0, 3, 2, 1).reshape(B, 64, 512)
    )
    const = {
        "wl2": np.ascontiguousarray(np.concatenate([wl, wl], axis=0).astype(bf)),
        "wt_hp": np.ascontiguousarray(np.concatenate([wp.T, wr.T], axis=0).astype(bf)),
        "wt_hr": np.ascontiguousarray(np.concatenate([wr.T, wp.T], axis=0).astype(bf)),
        "whp_c": np.ascontiguousarray(whp.T.astype(bf)),
        "whr_c": np.ascontiguousarray(whr.T.astype(bf)),
        "ident": np.eye(128, dtype=np.float32),
    }
    return rev_bf, rev_t, post_bf, post_t, const


def run_on_hw(inputs: dict, trace: bool = False, **kw):
    nc = _get_nc()
    rev_bf, rev_t, post_bf, post_t, const = _prep_host_inputs(inputs)
    in_maps = []
    for c in range(NCORES):
        s = slice(c * B_LOC, (c + 1) * B_LOC)
        m = {
            "review_bf": np.ascontiguousarray(rev_bf[s]),
            "review_t": np.ascontiguousarray(rev_t[s]),
            "post_bf": np.ascontiguousarray(post_bf[s]),
            "post_t": np.ascontiguousarray(post_t[s]),
        }
        m.update(const)
        in_maps.append(m)
    res = run_bass_kernel_spmd(nc, in_maps, list(range(NCORES)), trace=trace, **kw)
    out = np.concatenate([res.results[c]["out"] for c in range(NCORES)], axis=0)
    return out, res


def kernel(**inputs) -> np.ndarray:
    out, _ = run_on_hw(inputs, trace=False)
    return out.astype(np.float32)


# revision 36
# speedup vs baseline: 2.2169x; 1.2559x over previous
"""CoAttLayer Trainium2 kernel — pure data-parallel over batch on 8 NeuronCores.

Reference computation (per batch element b, T=1024, N=512, D=64, K=80):
  L  = tanh(R @ Wl @ P^T)                    (T, N)
  Hp = tanh(Wp @ P^T + (Wr @ R^T) @ L)       (K, N)
  Hr = tanh(Wr @ R^T + (Wp @ P^T) @ L^T)     (K, T)
  Ap = softmax(whp @ Hp), Ar = softmax(whr @ Hr)
  out[b] = concat(P^T @ Ap, R^T @ Ar)        (2D,)

Reassociated into D-sized contractions:
  Hp = [Wp | Wr] @ [P^T ; X]   with X = R^T @ L    (D, N)
  Hr = [Wr | Wp] @ [R^T ; Y]   with Y = P^T @ L^T  (D, T)

Design notes (from trace analysis):
 - The PE HAM clock governor only counts real matmul activity; transpose-mode
   instructions poison it back to 1.2 GHz. So the batch loop contains ZERO PE
   transposes: all static transposed layouts (R^T, P^T, weight stacks) are
   prepared on the HOST, and the data-dependent L^T is produced by bouncing
   L through DRAM and reading it back through the DMA xbar transpose engine
   (~180 GB/s, fully off the compute engines).
 - All matmul operands are bf16 (fp32 PSUM accumulate); tanh lives on the
   Scalar engine with 1024-wide evacuations; PSUM evacuations go to DVE.
 - Softmax is batched across the 8 local batch elements on partitions.
"""

import numpy as np

import concourse.bass as bass
import concourse.bacc as bacc
import concourse.mybir as mybir
import concourse.tile as tile
from concourse.bass_utils import run_bass_kernel_spmd

F32 = mybir.dt.float32
BF16 = mybir.dt.bfloat16
AF = mybir.ActivationFunctionType

B_LOC = 8      # batch elements per core
T, N, D, K = 1024, 512, 64, 80
TI = T // 128  # 8 t-tiles
NI = N // 128  # 4 n-tiles
NCORES = 8


def build_kernel():
    nc = bacc.Bacc("TRN2", debug=False, target_bir_lowering=False)

    ins = {}
    for name, shape, dt in [
        ("review_bf", [B_LOC, T, D], BF16),
        ("review_t", [B_LOC, D, T], BF16),
        ("post_bf", [B_LOC, N, D], BF16),
        ("post_t", [B_LOC, D, N], BF16),
        ("wl2", [2 * D, D], BF16),
        ("wt_hp", [2 * D, K], BF16),
        ("wt_hr", [2 * D, K], BF16),
        ("whp_c", [K, 1], BF16),
        ("whr_c", [K, 1], BF16),
        ("ident", [128, 128], F32),
    ]:
        ins[name] = nc.declare_dram_parameter(name, shape, dt, isOutput=False)
    out_e = nc.declare_dram_parameter("out", [B_LOC, 2 * D], F32, isOutput=True)

    with tile.TileContext(nc) as tc:
        _body(nc, tc, ins, out_e)

    nc.compile()
    return nc


def _body(nc, tc, ins, out_e):
    from contextlib import ExitStack

    ctx = ExitStack()
    cpool = ctx.enter_context(tc.tile_pool(name="const", bufs=1))
    inpool = ctx.enter_context(tc.tile_pool(name="inputs", bufs=1))
    wk = ctx.enter_context(tc.tile_pool(name="work", bufs=2))
    ps_mm = ctx.enter_context(tc.tile_pool(name="ps_mm", bufs=2, space="PSUM"))
    ps_acc = ctx.enter_context(tc.tile_pool(name="ps_acc", bufs=3, space="PSUM"))

    # ---------------- constants (all pre-transposed on host) ----------------
    ident_f = cpool.tile([128, 128], F32)
    nc.sync.dma_start(out=ident_f[:], in_=ins["ident"].ap())
    ident_b = cpool.tile([128, 128], BF16)
    nc.vector.tensor_copy(ident_b[:], ident_f[:])
    wl2 = cpool.tile([2 * D, D], BF16)
    nc.sync.dma_start(out=wl2[:], in_=ins["wl2"].ap())
    wt_hp = cpool.tile([2 * D, K], BF16)
    nc.sync.dma_start(out=wt_hp[:], in_=ins["wt_hp"].ap())
    wt_hr = cpool.tile([2 * D, K], BF16)
    nc.sync.dma_start(out=wt_hr[:], in_=ins["wt_hr"].ap())
    whp_b = cpool.tile([K, 1], BF16)
    nc.sync.dma_start(out=whp_b[:], in_=ins["whp_c"].ap())
    whr_b = cpool.tile([K, 1], BF16)
    nc.sync.dma_start(out=whr_b[:], in_=ins["whr_c"].ap())

    # Persistent bf16 inputs (written once by merged DMAs, then read-only)
    r_ball = inpool.tile([128, B_LOC, TI, D], BF16)
    p_ball = inpool.tile([128, B_LOC, NI, D], BF16)

    # Per-batch logits, transposed layout: cols 0:4 ap n-tiles, 4:12 ar t-tiles
    lgt_all = inpool.tile([128, 12, B_LOC], F32)

    # ---------------- main compute, two global phases ----------------
    # Phase 1 (per batch): loads, RlT, L (+tanh), X, L->DRAM, LT xbar reads.
    # Phase 2 (per batch): Hp, Y, Hr, logits — consumes the LT tiles whose
    # DMA-transpose latency was hidden behind the rest of phase 1.
    # K=64 matmuls are packed two-per-issue into disjoint PE row groups
    # (K<=64 streams at half rate unpacked: 427 vs 117 ns per N=512 matmul).
    lt_pool = ctx.enter_context(tc.tile_pool(name="lt", bufs=B_LOC))
    ps_tp = ctx.enter_context(tc.tile_pool(name="ps_tp", bufs=1, space="PSUM"))
    st_all = [dict() for _ in range(B_LOC)]

    # Merged input loads: one HWDGE trigger per tensor (the per-trigger cost
    # on the in-order Sync sequencer is ~0.7us — keep the count tiny).
    hr_all = inpool.tile([128, B_LOC, T], BF16)
    hp_all = inpool.tile([128, B_LOC, N], BF16)
    rev_v = ins["review_bf"].ap().rearrange("b (p i) d -> p b i d", i=TI)
    post_v = ins["post_bf"].ap().rearrange("b (p j) d -> p b j d", j=NI)
    rt_v = ins["review_t"].ap().rearrange("b d t -> d b t")
    pt_v = ins["post_t"].ap().rearrange("b d t -> d b t")
    # batch-0 inputs first (compute gates on them), then the rest merged
    for lo, hi in ((0, 1), (1, B_LOC)):
        s = slice(lo, hi)
        for h in range(2):
            nc.sync.dma_start(out=hr_all[h * D:(h + 1) * D, s, :], in_=rt_v[:, s])
            nc.sync.dma_start(out=hp_all[h * D:(h + 1) * D, s, :], in_=pt_v[:, s])
        nc.sync.dma_start(out=r_ball[:, s], in_=rev_v[:, s])
        nc.sync.dma_start(out=p_ball[:, s], in_=post_v[:, s])

    def phase1(b):
        st = st_all[b]
        st["hr_in"] = hr_all[:, b, :]
        st["hp_in"] = hp_all[:, b, :]
        st["rlt2"] = wk.tile([128, N], BF16, tag="rlt2", name=f"rlt2{b}")
        l_sb = wk.tile([128, TI, N], BF16, tag="l_sb", name=f"l_sb{b}")
        st["lt_sb"] = lt_pool.tile([128, NI, T], BF16, tag="lt", name=f"lt_sb{b}")
        lps = {}

        # rlt2 layout: top half = RlT chunks 0,2,4,6; bottom = 1,3,5,7,
        # one packed pair with even/odd interleaved views of replicated Rt.
        pss = []
        for h in range(2):
            ps = ps_mm.tile([D, 512], F32, tag="mm", name=f"rlt_ps{b}_{h}")
            rt_v = st["hr_in"][h * D:(h + 1) * D, :].rearrange(
                "p (c two k) -> p two c k", two=2, k=128
            )[:, h]
            nc.tensor.matmul(
                ps[:], wl2[h * D:(h + 1) * D, :], rt_v, tile_position=(h * D, 0)
            )
            pss.append(ps)
        for h in range(2):
            nc.vector.tensor_copy(st["rlt2"][h * D:(h + 1) * D, :], pss[h][:])

        def emit_l_pair(p):
            lp = ps_mm.tile([128, 2, N], F32, tag="mm", name=f"lps{b}_{p}")
            lps[p] = lp
            for h in range(2):
                nc.tensor.matmul(
                    lp[:, h],
                    st["rlt2"][h * D:(h + 1) * D, p * 128:(p + 1) * 128],
                    st["hp_in"][h * D:(h + 1) * D, :],
                    tile_position=(h * D, 0),
                )

        def emit_l_evac(p):
            nc.scalar.activation(l_sb[:, 2 * p:2 * p + 2, :], lps[p][:], AF.Tanh)

        def emit_lt_pair(p):
            # PE block-transposes of the tanh'd pair into one 1-bank PSUM
            # tile, then a single wide DVE evacuation into lt_sb.
            tp = ps_tp.tile([128, NI, 2, 128], BF16, tag="tp", name=f"tp{b}_{p}")
            for j in range(NI):
                for h in range(2):
                    nc.tensor.transpose(
                        tp[:, j, h],
                        l_sb[:, 2 * p + h, j * 128:(j + 1) * 128],
                        ident_b[:],
                    )
            nc.vector.tensor_copy(
                st["lt_sb"][:, :, 2 * p * 128:(2 * p + 2) * 128]
                .rearrange("q j (two k) -> q j two k", k=128),
                tp[:],
            )

        xps = ps_acc.tile([D, N], F32, tag="acc", name=f"xps{b}")
        emit_l_pair(0)
        emit_l_pair(1)
        emit_l_evac(0)
        for p in range(TI // 2):
            for i in (2 * p, 2 * p + 1):
                nc.tensor.matmul(
                    xps[:], r_ball[:, b, i], l_sb[:, i],
                    start=(i == 0), stop=(i == TI - 1),
                )
            if p + 2 < TI // 2:
                emit_l_pair(p + 2)
            if p + 1 < TI // 2:
                emit_l_evac(p + 1)
            emit_lt_pair(p)
        nc.vector.tensor_copy(st["hp_in"][D:128, :], xps[:])

    def phase2(b):
        st = st_all[b]
        hp_bf = wk.tile([K, N], BF16, tag="hp_bf", name=f"hp_bf{b}")
        hps = ps_acc.tile([K, N], F32, tag="acc", name=f"hps{b}")
        nc.tensor.matmul(hps[:], wt_hp[:], st["hp_in"][:])
        nc.scalar.activation(hp_bf[:], hps[:], AF.Tanh)

        yps = [
            ps_acc.tile([D, 512], F32, tag="acc", name=f"yps{b}_{c}")
            for c in range(2)
        ]
        for c in range(2):
            for j in range(NI):
                nc.tensor.matmul(
                    yps[c][:], p_ball[:, b, j],
                    st["lt_sb"][:, j, c * 512:(c + 1) * 512],
                    start=(j == 0), stop=(j == NI - 1),
                )
            nc.vector.tensor_copy(
                st["hr_in"][D:128, c * 512:(c + 1) * 512], yps[c][:]
            )

        hr_bf = wk.tile([K, T], BF16, tag="hr_bf", name=f"hr_bf{b}")
        for c in range(2):
            hrs = ps_acc.tile([K, 512], F32, tag="acc", name=f"hrs{b}_{c}")
            nc.tensor.matmul(hrs[:], wt_hr[:], st["hr_in"][:, c * 512:(c + 1) * 512])
            nc.scalar.activation(hr_bf[:, c * 512:(c + 1) * 512], hrs[:], AF.Tanh)

        lg_ps = ps_acc.tile([128, 12], F32, tag="acc", name=f"lg_ps{b}")
        for j in range(NI):
            nc.tensor.matmul(
                lg_ps[:, j:j + 1], hp_bf[:, j * 128:(j + 1) * 128], whp_b[:],
                skip_group_check=True,
            )
        for i in range(TI):
            nc.tensor.matmul(
                lg_ps[:, 4 + i:5 + i], hr_bf[:, i * 128:(i + 1) * 128], whr_b[:],
                skip_group_check=True,
            )
        nc.vector.tensor_copy(lgt_all[:, :, b], lg_ps[:])

    G = B_LOC // 2
    NG = B_LOC // G
    logits = [inpool.tile([G, 12 * 128], F32, name=f"logits{g}") for g in range(NG)]
    probs = [inpool.tile([G, 12 * 128], F32, name=f"probs{g}") for g in range(NG)]
    pn = [inpool.tile([G, 12 * 128], F32, name=f"pn{g}") for g in range(NG)]
    mx = [inpool.tile([G, 2], F32, name=f"mx{g}") for g in range(NG)]
    nmx = [inpool.tile([G, 2], F32, name=f"nmx{g}") for g in range(NG)]
    sums = [inpool.tile([G, 2], F32, name=f"sums{g}") for g in range(NG)]
    rcp = [inpool.tile([G, 2], F32, name=f"rcp{g}") for g in range(NG)]
    prt = inpool.tile([128, 12, B_LOC], BF16)
    co_sb = inpool.tile([D, 2, B_LOC], F32)

    def smpool(g):
        gs = slice(g * G, (g + 1) * G)
        lgits, prbs, pnn = logits[g], probs[g], pn[g]
        mxx, nmxx, summ, rcpp = mx[g], nmx[g], sums[g], rcp[g]
        # transpose this group's logits into (G, 1536) rows
        for gg in range(3):
            lgt_t_ps = ps_acc.tile([G, 512], F32, tag="acc", name=f"lgt{g}_{gg}")
            for jj in range(4):
                j = gg * 4 + jj
                nc.tensor.transpose(
                    lgt_t_ps[:, jj * 128:(jj + 1) * 128],
                    lgt_all[:, j, gs],
                    ident_f[:],
                )
            nc.vector.tensor_copy(lgits[:, gg * 512:(gg + 1) * 512], lgt_t_ps[:])

        # logits are bounded (|whp|_1-weighted tanh values), so exp cannot
        # overflow — softmax without the max-subtraction pass.
        nc.scalar.activation(
            prbs[:, 0:N], lgits[:, 0:N], AF.Exp, accum_out=summ[:, 0:1]
        )
        nc.scalar.activation(
            prbs[:, N:N + T], lgits[:, N:N + T], AF.Exp, accum_out=summ[:, 1:2]
        )
        nc.vector.reciprocal(rcpp[:, :], summ[:, :])
        nc.vector.tensor_scalar_mul(pnn[:, 0:N], prbs[:, 0:N], rcpp[:, 0:1])
        nc.vector.tensor_scalar_mul(
            pnn[:, N:N + T], prbs[:, N:N + T], rcpp[:, 1:2]
        )
        prt_ps = ps_acc.tile([128, 12 * G], F32, tag="acc", name=f"prt{g}")
        for j in range(12):
            nc.tensor.transpose(
                prt_ps[:, j * G:(j + 1) * G],
                pnn[:, j * 128:(j + 1) * 128],
                ident_f[0:G, 0:G],
            )
        nc.vector.tensor_copy(prt[:, :, gs], prt_ps[:])
        co_ps = ps_acc.tile([D, 2, G], F32, tag="acc", name=f"co_ps{g}")
        for bb in range(G):
            b = g * G + bb
            for j in range(NI):
                nc.tensor.matmul(
                    co_ps[:, 0, bb:bb + 1], p_ball[:, b, j], prt[:, j, b:b + 1],
                    start=(j == 0), stop=(j == NI - 1), skip_group_check=True,
                )
            for i in range(TI):
                nc.tensor.matmul(
                    co_ps[:, 1, bb:bb + 1], r_ball[:, b, i],
                    prt[:, 4 + i, b:b + 1],
                    start=(i == 0), stop=(i == TI - 1), skip_group_check=True,
                )
        nc.vector.tensor_copy(co_sb[:, :, gs], co_ps[:])

    phase1(0)
    for b in range(1, B_LOC):
        k = b - 1
        phase2(k)
        phase1(b)
        if k == G - 1:
            smpool(0)
    phase2(B_LOC - 1)
    smpool(1)

    # Transpose (64, 16) -> (16, 64); row h*8+b is the h-half of out[b]
    cot_ps = ps_acc.tile([2 * B_LOC, D], F32, tag="acc")
    nc.tensor.transpose(
        cot_ps[:], co_sb[:].rearrange("d h b -> d (h b)"), ident_f[0:D, 0:D]
    )
    out_sb = inpool.tile([2 * B_LOC, D], F32)
    nc.vector.tensor_copy(out_sb[:], cot_ps[:])
    nc.sync.dma_start(out=out_e.ap()[:, 0:D], in_=out_sb[0:B_LOC, :])
    nc.sync.dma_start(out=out_e.ap()[:, D:2 * D], in_=out_sb[B_LOC:2 * B_LOC, :])
    ctx.close()


# revision 37
# speedup vs baseline: 2.3180x; 1.0456x over previous
"""CoAttLayer Trainium2 kernel — pure data-parallel over batch on 8 NeuronCores.

Reference computation (per batch element b, T=1024, N=512, D=64, K=80):
  L  = tanh(R @ Wl @ P^T)                    (T, N)
  Hp = tanh(Wp @ P^T + (Wr @ R^T) @ L)       (K, N)
  Hr = tanh(Wr @ R^T + (Wp @ P^T) @ L^T)     (K, T)
  Ap = softmax(whp @ Hp), Ar = softmax(whr @ Hr)
  out[b] = concat(P^T @ Ap, R^T @ Ar)        (2D,)

Reassociated into D-sized contractions:
  Hp = [Wp | Wr] @ [P^T ; X]   with X = R^T @ L    (D, N)
  Hr = [Wr | Wp] @ [R^T ; Y]   with Y = P^T @ L^T  (D, T)

Design notes (from trace analysis):
 - The PE HAM clock governor only counts real matmul activity; transpose-mode
   instructions poison it back to 1.2 GHz. So the batch loop contains ZERO PE
   transposes: all static transposed layouts (R^T, P^T, weight stacks) are
   prepared on the HOST, and the data-dependent L^T is produced by bouncing
   L through DRAM and reading it back through the DMA xbar transpose engine
   (~180 GB/s, fully off the compute engines).
 - All matmul operands are bf16 (fp32 PSUM accumulate); tanh lives on the
   Scalar engine with 1024-wide evacuations; PSUM evacuations go to DVE.
 - Softmax is batched across the 8 local batch elements on partitions.
"""

import numpy as np

import concourse.bass as bass
import concourse.bacc as bacc
import concourse.mybir as mybir
import concourse.tile as tile
from concourse.bass_utils import run_bass_kernel_spmd

F32 = mybir.dt.float32
BF16 = mybir.dt.bfloat16
AF = mybir.ActivationFunctionType

B_LOC = 8      # batch elements per core
T, N, D, K = 1024, 512, 64, 80
TI = T // 128  # 8 t-tiles
NI = N // 128  # 4 n-tiles
NCORES = 8


def build_kernel():
    nc = bacc.Bacc("TRN2", debug=False, target_bir_lowering=False)

    ins = {}
    for name, shape, dt in [
        ("review_bf", [B_LOC, T, D], BF16),
        ("review_t", [B_LOC, D, T], BF16),
        ("post_bf", [B_LOC, N, D], BF16),
        ("post_t", [B_LOC, D, N], BF16),
        ("wl2", [2 * D, D], BF16),
        ("wt_hp", [2 * D, K], BF16),
        ("wt_hr", [2 * D, K], BF16),
        ("whp_c", [K, 1], BF16),
        ("whr_c", [K, 1], BF16),
        ("ident", [128, 128], F32),
    ]:
        ins[name] = nc.declare_dram_parameter(name, shape, dt, isOutput=False)
    out_e = nc.declare_dram_parameter("out", [B_LOC, 2 * D], F32, isOutput=True)

    with tile.TileContext(nc) as tc:
        _body(nc, tc, ins, out_e)

    nc.compile()
    return nc


def _body(nc, tc, ins, out_e):
    from contextlib import ExitStack

    ctx = ExitStack()
    cpool = ctx.enter_context(tc.tile_pool(name="const", bufs=1))
    inpool = ctx.enter_context(tc.tile_pool(name="inputs", bufs=1))
    wk = ctx.enter_context(tc.tile_pool(name="work", bufs=2))
    ps_mm = ctx.enter_context(tc.tile_pool(name="ps_mm", bufs=2, space="PSUM"))
    ps_acc = ctx.enter_context(tc.tile_pool(name="ps_acc", bufs=2, space="PSUM"))

    # ---------------- constants (all pre-transposed on host) ----------------
    ident_f = cpool.tile([128, 128], F32)
    nc.sync.dma_start(out=ident_f[:], in_=ins["ident"].ap())
    ident_b = cpool.tile([128, 128], BF16)
    nc.vector.tensor_copy(ident_b[:], ident_f[:])
    wl2 = cpool.tile([2 * D, D], BF16)
    nc.sync.dma_start(out=wl2[:], in_=ins["wl2"].ap())
    wt_hp = cpool.tile([2 * D, K], BF16)
    nc.sync.dma_start(out=wt_hp[:], in_=ins["wt_hp"].ap())
    wt_hr = cpool.tile([2 * D, K], BF16)
    nc.sync.dma_start(out=wt_hr[:], in_=ins["wt_hr"].ap())
    whp_b = cpool.tile([K, 1], BF16)
    nc.sync.dma_start(out=whp_b[:], in_=ins["whp_c"].ap())
    whr_b = cpool.tile([K, 1], BF16)
    nc.sync.dma_start(out=whr_b[:], in_=ins["whr_c"].ap())

    # Persistent bf16 inputs (written once by merged DMAs, then read-only)
    r_ball = inpool.tile([128, B_LOC, TI, D], BF16)
    p_ball = inpool.tile([128, B_LOC, NI, D], BF16)

    # Per-batch logits, transposed layout: cols 0:4 ap n-tiles, 4:12 ar t-tiles
    lgt_all = inpool.tile([128, 12, B_LOC], F32)

    # ---------------- main compute, two global phases ----------------
    # Phase 1 (per batch): loads, RlT, L (+tanh), X, L->DRAM, LT xbar reads.
    # Phase 2 (per batch): Hp, Y, Hr, logits — consumes the LT tiles whose
    # DMA-transpose latency was hidden behind the rest of phase 1.
    # K=64 matmuls are packed two-per-issue into disjoint PE row groups
    # (K<=64 streams at half rate unpacked: 427 vs 117 ns per N=512 matmul).
    lt_pool = ctx.enter_context(tc.tile_pool(name="lt", bufs=B_LOC))
    ps_tp = ctx.enter_context(tc.tile_pool(name="ps_tp", bufs=2, space="PSUM"))
    st_all = [dict() for _ in range(B_LOC)]

    # Merged input loads: one HWDGE trigger per tensor (the per-trigger cost
    # on the in-order Sync sequencer is ~0.7us — keep the count tiny).
    hr_all = inpool.tile([128, B_LOC, T], BF16)
    hp_all = inpool.tile([128, B_LOC, N], BF16)
    rev_v = ins["review_bf"].ap().rearrange("b (p i) d -> p b i d", i=TI)
    post_v = ins["post_bf"].ap().rearrange("b (p j) d -> p b j d", j=NI)
    rt_v = ins["review_t"].ap().rearrange("b d t -> d b t")
    pt_v = ins["post_t"].ap().rearrange("b d t -> d b t")
    # batch-0 inputs first (compute gates on them), then the rest merged
    for lo, hi in ((0, 1), (1, B_LOC)):
        s = slice(lo, hi)
        for h in range(2):
            nc.sync.dma_start(out=hr_all[h * D:(h + 1) * D, s, :], in_=rt_v[:, s])
            nc.sync.dma_start(out=hp_all[h * D:(h + 1) * D, s, :], in_=pt_v[:, s])
        nc.sync.dma_start(out=r_ball[:, s], in_=rev_v[:, s])
        nc.sync.dma_start(out=p_ball[:, s], in_=post_v[:, s])

    def phase1(b):
        st = st_all[b]
        st["hr_in"] = hr_all[:, b, :]
        st["hp_in"] = hp_all[:, b, :]
        st["rlt2"] = wk.tile([128, N], BF16, tag="rlt2", name=f"rlt2{b}")
        l_sb = wk.tile([128, TI, N], BF16, tag="l_sb", name=f"l_sb{b}")
        st["lt_sb"] = lt_pool.tile([128, NI, T], BF16, tag="lt", name=f"lt_sb{b}")
        lps = {}

        # rlt2 layout: top half = RlT chunks 0,2,4,6; bottom = 1,3,5,7,
        # one packed pair with even/odd interleaved views of replicated Rt.
        pss = []
        for h in range(2):
            ps = ps_mm.tile([D, 512], F32, tag="mm", name=f"rlt_ps{b}_{h}")
            rt_v = st["hr_in"][h * D:(h + 1) * D, :].rearrange(
                "p (c two k) -> p two c k", two=2, k=128
            )[:, h]
            nc.tensor.matmul(
                ps[:], wl2[h * D:(h + 1) * D, :], rt_v, tile_position=(h * D, 0)
            )
            pss.append(ps)
        for h in range(2):
            nc.scalar.copy(st["rlt2"][h * D:(h + 1) * D, :], pss[h][:])

        def emit_l_pair(p):
            lp = ps_mm.tile([128, 2, N], F32, tag="mm", name=f"lps{b}_{p}")
            lps[p] = lp
            for h in range(2):
                nc.tensor.matmul(
                    lp[:, h],
                    st["rlt2"][h * D:(h + 1) * D, p * 128:(p + 1) * 128],
                    st["hp_in"][h * D:(h + 1) * D, :],
                    tile_position=(h * D, 0),
                )

        def emit_l_evac(p):
            nc.scalar.activation(l_sb[:, 2 * p:2 * p + 2, :], lps[p][:], AF.Tanh)

        def emit_lt_pair(p):
            # PE block-transposes of the tanh'd pair into one 1-bank PSUM
            # tile, then a single wide DVE evacuation into lt_sb.
            tp = ps_tp.tile([128, NI, 2, 128], BF16, tag="tp", name=f"tp{b}_{p}")
            for j in range(NI):
                for h in range(2):
                    nc.tensor.transpose(
                        tp[:, j, h],
                        l_sb[:, 2 * p + h, j * 128:(j + 1) * 128],
                        ident_b[:],
                    )
            nc.vector.tensor_copy(
                st["lt_sb"][:, :, 2 * p * 128:(2 * p + 2) * 128]
                .rearrange("q j (two k) -> q j two k", k=128),
                tp[:],
            )

        xps = ps_acc.tile([D, N], F32, tag="acc", name=f"xps{b}")
        emit_l_pair(0)
        emit_l_pair(1)
        emit_l_evac(0)
        for p in range(TI // 2):
            for i in (2 * p, 2 * p + 1):
                nc.tensor.matmul(
                    xps[:], r_ball[:, b, i], l_sb[:, i],
                    start=(i == 0), stop=(i == TI - 1),
                )
            if p + 2 < TI // 2:
                emit_l_pair(p + 2)
            if p + 1 < TI // 2:
                emit_l_evac(p + 1)
            emit_lt_pair(p)
        nc.vector.tensor_copy(st["hp_in"][D:128, :], xps[:])

    def phase2(b):
        st = st_all[b]
        hp_bf = wk.tile([K, N], BF16, tag="hp_bf", name=f"hp_bf{b}")
        hps = ps_acc.tile([K, N], F32, tag="acc", name=f"hps{b}")
        nc.tensor.matmul(hps[:], wt_hp[:], st["hp_in"][:])
        nc.scalar.activation(hp_bf[:], hps[:], AF.Tanh)

        yps = [
            ps_acc.tile([D, 512], F32, tag="acc", name=f"yps{b}_{c}")
            for c in range(2)
        ]
        for c in range(2):
            for j in range(NI):
                nc.tensor.matmul(
                    yps[c][:], p_ball[:, b, j],
                    st["lt_sb"][:, j, c * 512:(c + 1) * 512],
                    start=(j == 0), stop=(j == NI - 1),
                )
            nc.vector.tensor_copy(
                st["hr_in"][D:128, c * 512:(c + 1) * 512], yps[c][:]
            )

        hr_bf = wk.tile([K, T], BF16, tag="hr_bf", name=f"hr_bf{b}")
        for c in range(2):
            hrs = ps_acc.tile([K, 512], F32, tag="acc", name=f"hrs{b}_{c}")
            nc.tensor.matmul(hrs[:], wt_hr[:], st["hr_in"][:, c * 512:(c + 1) * 512])
            nc.scalar.activation(hr_bf[:, c * 512:(c + 1) * 512], hrs[:], AF.Tanh)

        lg_ps = ps_acc.tile([128, 12], F32, tag="acc", name=f"lg_ps{b}")
        for j in range(NI):
            nc.tensor.matmul(
                lg_ps[:, j:j + 1], hp_bf[:, j * 128:(j + 1) * 128], whp_b[:],
                skip_group_check=True,
            )
        for i in range(TI):
            nc.tensor.matmul(
                lg_ps[:, 4 + i:5 + i], hr_bf[:, i * 128:(i + 1) * 128], whr_b[:],
                skip_group_check=True,
            )
        nc.vector.tensor_copy(lgt_all[:, :, b], lg_ps[:])

    G = B_LOC // 2
    NG = B_LOC // G
    logits = [inpool.tile([G, 12 * 128], F32, name=f"logits{g}") for g in range(NG)]
    probs = [inpool.tile([G, 12 * 128], F32, name=f"probs{g}") for g in range(NG)]
    pn = [inpool.tile([G, 12 * 128], F32, name=f"pn{g}") for g in range(NG)]
    mx = [inpool.tile([G, 2], F32, name=f"mx{g}") for g in range(NG)]
    nmx = [inpool.tile([G, 2], F32, name=f"nmx{g}") for g in range(NG)]
    sums = [inpool.tile([G, 2], F32, name=f"sums{g}") for g in range(NG)]
    rcp = [inpool.tile([G, 2], F32, name=f"rcp{g}") for g in range(NG)]
    prt = inpool.tile([128, 12, B_LOC], BF16)
    co_sb = inpool.tile([D, 2, B_LOC], F32)

    def smpool(g):
        gs = slice(g * G, (g + 1) * G)
        lgits, prbs, pnn = logits[g], probs[g], pn[g]
        mxx, nmxx, summ, rcpp = mx[g], nmx[g], sums[g], rcp[g]
        # transpose this group's logits into (G, 1536) rows
        for gg in range(3):
            lgt_t_ps = ps_acc.tile([G, 512], F32, tag="acc", name=f"lgt{g}_{gg}")
            for jj in range(4):
                j = gg * 4 + jj
                nc.tensor.transpose(
                    lgt_t_ps[:, jj * 128:(jj + 1) * 128],
                    lgt_all[:, j, gs],
                    ident_f[:],
                )
            nc.vector.tensor_copy(lgits[:, gg * 512:(gg + 1) * 512], lgt_t_ps[:])

        # logits are bounded (|whp|_1-weighted tanh values), so exp cannot
        # overflow — softmax without the max-subtraction pass.
        nc.scalar.activation(
            prbs[:, 0:N], lgits[:, 0:N], AF.Exp, accum_out=summ[:, 0:1]
        )
        nc.scalar.activation(
            prbs[:, N:N + T], lgits[:, N:N + T], AF.Exp, accum_out=summ[:, 1:2]
        )
        nc.vector.reciprocal(rcpp[:, :], summ[:, :])
        nc.vector.tensor_scalar_mul(pnn[:, 0:N], prbs[:, 0:N], rcpp[:, 0:1])
        nc.vector.tensor_scalar_mul(
            pnn[:, N:N + T], prbs[:, N:N + T], rcpp[:, 1:2]
        )
        prt_ps = ps_acc.tile([128, 12 * G], F32, tag="acc", name=f"prt{g}")
        for j in range(12):
            nc.tensor.transpose(
                prt_ps[:, j * G:(j + 1) * G],
                pnn[:, j * 128:(j + 1) * 128],
                ident_f[0:G, 0:G],
            )
        nc.vector.tensor_copy(prt[:, :, gs], prt_ps[:])
        co_ps = ps_acc.tile([D, 2, G], F32, tag="acc", name=f"co_ps{g}")
        for bb in range(G):
            b = g * G + bb
            for j in range(NI):
                nc.tensor.matmul(
                    co_ps[:, 0, bb:bb + 1], p_ball[:, b, j], prt[:, j, b:b + 1],
                    start=(j == 0), stop=(j == NI - 1), skip_group_check=True,
                )
            for i in range(TI):
                nc.tensor.matmul(
                    co_ps[:, 1, bb:bb + 1], r_ball[:, b, i],
                    prt[:, 4 + i, b:b + 1],
                    start=(i == 0), stop=(i == TI - 1), skip_group_check=True,
                )
        nc.vector.tensor_copy(co_sb[:, :, gs], co_ps[:])

    phase1(0)
    for b in range(1, B_LOC):
        k = b - 1
        phase2(k)
        phase1(b)
        if k == G - 1:
            smpool(0)
    phase2(B_LOC - 1)
    smpool(1)

    # Transpose (64, 16) -> (16, 64); row h*8+b is the h-half of out[b]
    cot_ps = ps_acc.tile([2 * B_LOC, D], F32, tag="acc")
    nc.tensor.transpose(
        cot_ps[:], co_sb[:].rearrange("d h b -> d (h b)"), ident_f[0:D, 0:D]
    )
    out_sb = inpool.tile([2 * B_LOC, D], F32)
    nc.vector.tensor_copy(out_sb[:], cot_ps[:])
    nc.sync.dma_start(out=out_e.ap()[:, 0:D], in_=out_sb[0:B_LOC, :])
    nc.sync.dma_start(out=out_e.ap()[:, D:2 * D], in_=out_sb[B_LOC:2 * B_LOC, :])
    ctx.close()
